# revision 43
# baseline (speedup 1.0000x reference)
"""Trainium2 Bass kernel: decoder multi-head attention (B=2, S=2048, D=1024, 16 heads).

Sharding: 8 cores = 2 batches x 4 head-groups (4 heads / 256 dims per core).
Per core (batch b, head group hg), all in transposed layouts:
  Q^T = (wq_c)^T @ xq[b]^T + bq_c      [256, 2048]
  K^T = (wk_c)^T @ xk[b]^T             [256, 2048]
  V   = xv[b] @ wv_c                   [2048, 256] token-major (no bias)
  per head h: scores^T[sk,sq] = K_h^T.T @ Q_h^T            (K=64, head pairs
              run concurrently on PE row groups 0-63/64-127)
              P^T = exp(scores^T/8) * mask^T               (fp16)
              [U^T; rowsum] = [V_h | 1].T @ P^T            (ones col -> rowsum)
  per pair p: UT2_p[128,sq] = [U_h0; U_h1] * (1/rowsum)    (recip_approx_fast
              on PSUM row 64 + gpsimd partition_broadcast; no DRAM roundtrip)
  y_partial = sum_p UT2_p.T @ wf2_p                        [2048, 1024]
              (head-pair stacked: contraction 128)
Host: out[b] = sum_hg y_partial + bv @ wf + bf
(v bias folded out: attention rows sum to 1, so attn@(V+bv) = attn@V + bv.)

Scheduling: x tiles DMA'd once per token group (shared by both w row tiles);
all mask DMAs issued up front on the gpsimd queue; PSUM evictions on DVE
(ACT does exp only); q-proj for the second q-block and fc for the previous
q-block ride in the attention pair-boundary bubbles.
"""

import sys

if "/opt/trn_rl_repo" not in sys.path:
    sys.path.insert(0, "/opt/trn_rl_repo")

import numpy as np

B, S, D = 2, 2048, 1024
NH, DK = 16, 64
NCORES = 8
HPC = 4            # heads per core
HD = HPC * DK      # 256 head dims per core
QB = 1024          # q-block (free dim of scores^T tiles)
NQB = S // QB      # 2
NKT = S // 128     # 16 sk tiles
KC = D // 128      # 8 contraction chunks for projections
LAG = 3            # umm trails scores by LAG sk-tiles

_CACHE = {}


def _build():
    import concourse.mybir as mybir
    import concourse.tile as tile
    from concourse import bacc

    f32 = mybir.dt.float32
    f16 = mybir.dt.float16
    EXP = mybir.ActivationFunctionType.Exp

    nc = bacc.Bacc(
        "TRN2",
        target_bir_lowering=False,
        debug=False,
        enable_asserts=False,
        num_devices=NCORES,
    )

    xq_d = nc.dram_tensor("xqT", [128, S // 256, KC, 256], f16, kind="ExternalInput")
    xk_d = nc.dram_tensor("xkT", [128, S // 256, KC, 256], f16, kind="ExternalInput")
    xv_d = nc.dram_tensor("xvT", [128, S // 256, KC, 256], f16, kind="ExternalInput")
    wq_d = nc.dram_tensor("wq", [128, KC, HD], f16, kind="ExternalInput")
    wk_d = nc.dram_tensor("wk", [128, KC, HD], f16, kind="ExternalInput")
    wv_d = nc.dram_tensor("wv", [128, KC, HD], f16, kind="ExternalInput")
    bq_d = nc.dram_tensor("bq", [128, 2], f32, kind="ExternalInput")
    wf_d = nc.dram_tensor("wf", [128, 2, D], f16, kind="ExternalInput")
    mk_d = nc.dram_tensor("maskT", [128, NQB, NKT, QB], f16, kind="ExternalInput")
    y_d = nc.dram_tensor("y", [S, D], f16, kind="ExternalOutput")

    with tile.TileContext(nc) as tc:
        with (
            tc.tile_pool(name="consts", bufs=1) as consts,
            tc.tile_pool(name="qk", bufs=1) as qkp,
            tc.tile_pool(name="maskc", bufs=5) as maskc,
            tc.tile_pool(name="xs", bufs=3) as xsp,
            tc.tile_pool(name="exp", bufs=5) as expp,
            tc.tile_pool(name="pt", bufs=10) as ptp,
            tc.tile_pool(name="usb", bufs=2) as usbp,
            tc.tile_pool(name="rs", bufs=2) as rsp,
            tc.tile_pool(name="bc", bufs=2) as bcp,
            tc.tile_pool(name="ut", bufs=4) as utp,
            tc.tile_pool(name="yo", bufs=2) as yop,
            tc.tile_pool(name="ps_s", bufs=2, space="PSUM") as ps_s,
            tc.tile_pool(name="ps_a", bufs=2, space="PSUM") as ps_a,
        ):
            # ---- constants (k/q first: the prologue needs only those) ----
            w_sb = {}
            for name, dram in (("k", wk_d), ("q", wq_d)):
                t = consts.tile([128, KC, HD], f16, tag=f"w{name}", name=f"w{name}")
                nc.sync.dma_start(out=t, in_=dram[:])
                w_sb[name] = t
            bq_sb = consts.tile([128, 2], f32, tag="bq")
            nc.sync.dma_start(out=bq_sb, in_=bq_d[:])

            # ---- mask tiles: qb0 right after the prologue; qb1 during
            # qb0-pair1 (issued on the gpsimd queue) ----
            mtiles = {}

            def mask_load(qb, grp):
                mg = maskc.tile([128, 4, QB], f16, tag="mask", name=f"m{qb}_{grp}")
                nc.gpsimd.dma_start(
                    out=mg, in_=mk_d[:][:, qb, 4 * grp : 4 * grp + 4, :]
                )
                mtiles[(qb, grp)] = mg

            # persistent activations
            QT = [qkp.tile([128, S], f16, tag=f"qt{m}", name=f"qt{m}") for m in range(2)]
            KT = [qkp.tile([128, S], f16, tag=f"kt{m}", name=f"kt{m}") for m in range(2)]
            V = [qkp.tile([128, HPC * 65], f16, tag=f"v{st}", name=f"v{st}") for st in range(NKT)]

            # ---------- emission units ----------
            def proj_dma(proj, g):
                """Issue one token group's 1MB x DMA; returns the tile."""
                src_d = {"q": xq_d, "k": xk_d, "v": xv_d}[proj]
                xt2 = xsp.tile([128, 2, KC, 256], f16, tag="xs", name="xt2")
                nc.sync.dma_start(out=xt2, in_=src_d[:][:, 2 * g : 2 * g + 2, :, :])
                return xt2

            def proj_qk_unit(proj, g, ms=(0, 1), xt2=None):
                """One token group (512 tokens) of q/k projection: one 1MB x
                DMA shared by the requested 128-row weight tiles; evictions
                on DVE."""
                if xt2 is None:
                    xt2 = proj_dma(proj, g)
                for m in ms:
                    for jj in range(2):
                        j = 2 * g + jj
                        xt = xt2[:, jj, :, :]
                        ps = ps_s.tile([128, QB], f32, tag="sc", name="ps")
                        for kc in range(KC):
                            nc.tensor.matmul(
                                ps[:, :256],
                                lhsT=w_sb[proj][:, kc, m * 128 : (m + 1) * 128],
                                rhs=xt[:, kc, :],
                                start=(kc == 0),
                                stop=(kc == KC - 1),
                            )
                        dst = (QT if proj == "q" else KT)[m][:, j * 256 : (j + 1) * 256]
                        if proj == "q":
                            nc.vector.tensor_scalar_add(dst, ps[:, :256], bq_sb[:, m : m + 1])
                        else:
                            nc.vector.tensor_copy(dst, ps[:, :256])

            def proj_v_unit(g, xt2=None):
                """One token group of v projection (4 sk tiles, token-major)."""
                if xt2 is None:
                    xt2 = proj_dma("v", g)
                for jj in range(2):
                    for sub in range(2):
                        st = (2 * g + jj) * 2 + sub
                        ps = ps_s.tile([128, QB], f32, tag="sc", name="ps")
                        for kc in range(KC):
                            nc.tensor.matmul(
                                ps[:, :HD],
                                lhsT=xt2[:, jj, kc, sub * 128 : (sub + 1) * 128],
                                rhs=w_sb["v"][:, kc, :],
                                start=(kc == 0),
                                stop=(kc == KC - 1),
                            )
                        vt = V[st]
                        vt3 = vt.rearrange("p (h c) -> p h c", h=HPC)
                        nc.gpsimd.memset(vt3[:, :, 64:65], 1.0)
                        nc.vector.tensor_copy(
                            vt3[:, :, 0:64],
                            ps[:, :HD].rearrange("p (h c) -> p h c", h=HPC),
                        )

            def scores(h, st, qb, pts):
                c, r = h // 2, 64 * (h % 2)
                ps = ps_s.tile([128, QB], f32, tag="sc", name="sc")
                for half in range(2):
                    nc.tensor.matmul(
                        ps[:, half * 512 : (half + 1) * 512],
                        lhsT=KT[c][r : r + 64, st * 128 : (st + 1) * 128],
                        rhs=QT[c][
                            r : r + 64,
                            qb * QB + half * 512 : qb * QB + (half + 1) * 512,
                        ],
                        start=True,
                        stop=True,
                    )
                et = expp.tile([128, QB], f16, tag="exp", name="et")
                nc.scalar.activation(et, ps, EXP, scale=0.125)
                pt = ptp.tile([128, QB], f16, tag="pt", name="pt")
                nc.vector.tensor_mul(pt, et, mtiles[(qb, st // 4)][:, st % 4, :])
                pts[(h, st)] = pt

            def umm(h, st, ups, pts):
                pt = pts.pop((h, st))
                up = ups[h]
                for half in range(2):
                    nc.tensor.matmul(
                        up[0:65, half * 512 : (half + 1) * 512],
                        lhsT=V[st][:, 65 * h : 65 * h + 65],
                        rhs=pt[:, half * 512 : (half + 1) * 512],
                        start=(st == 0),
                        stop=(st == NKT - 1),
                    )

            def norm_pair_evict(pair, ups, state):
                """Pair-end PSUM readout (frees the accumulators fast): U of
                each head into its own base-0 [64, QB] tile, rowsums into
                base-0 [1, QB] tiles. The reciprocal/broadcast/multiply is
                deferred (norm_pair_finish) to the next pair so it never
                blocks the DVE queue."""
                h0, h1 = 2 * pair, 2 * pair + 1
                up0, up1 = ups.pop(h0), ups.pop(h1)
                usb_lo = usbp.tile([64, QB], f32, tag="usb_lo", name="usb_lo")
                nc.vector.tensor_copy(usb_lo, up0[0:64, :])
                usb_hi = usbp.tile([64, QB], f32, tag="usb_hi", name="usb_hi")
                nc.vector.tensor_copy(usb_hi, up1[0:64, :])
                rs_a = rsp.tile([1, QB], f32, tag="rs_a", name="rs_a")
                nc.vector.tensor_copy(rs_a, up0[64:65, :])
                rs_b = rsp.tile([1, QB], f32, tag="rs_b", name="rs_b")
                nc.vector.tensor_copy(rs_b, up1[64:65, :])
                state["t"] = (usb_lo, usb_hi, rs_a, rs_b)

            def norm_pair_finish(gp, state, ut_pairs):
                """All on-chip: in-place approx reciprocal (base-0 custom op),
                gpsimd partition_broadcast into base-0 [64, QB] tiles, two DVE
                multiplies (both inputs base-0; only the output base differs)."""
                usb_lo, usb_hi, rs_a, rs_b = state.pop("t")
                nc.vector.reciprocal_approx_fast(out=rs_a, in_=rs_a)
                nc.vector.reciprocal_approx_fast(out=rs_b, in_=rs_b)
                bc0 = bcp.tile([64, QB], f32, tag="bc0", name="bc0")
                nc.gpsimd.partition_broadcast(bc0, rs_a)
                bc1 = bcp.tile([64, QB], f32, tag="bc1", name="bc1")
                nc.gpsimd.partition_broadcast(bc1, rs_b)
                ut2 = utp.tile([128, QB], f16, tag="ut", name="ut2")
                nc.vector.tensor_mul(ut2[0:64, :], usb_lo, bc0)
                nc.vector.tensor_mul(ut2[64:128, :], usb_hi, bc1)
                ut_pairs[gp] = ut2

            def fc_half(qb, g, jj, yo_box, ut_get):
                # one 128-row fc tile; on jj==1 ships the [256, D] yo tile
                if jj == 0:
                    yo_box["yo"] = yop.tile([128, 2, D], f16, tag="yo", name="yo")
                yo = yo_box["yo"]
                j = 2 * g + jj
                fp = ps_s.tile([128, QB], f32, tag="sc", name="fp")
                for half in range(2):
                    for p in range(2):
                        nc.tensor.matmul(
                            fp[:, half * 512 : (half + 1) * 512],
                            lhsT=ut_get(p)[:, j * 128 : (j + 1) * 128],
                            rhs=wf_sb[:, p, half * 512 : (half + 1) * 512],
                            start=(p == 0),
                            stop=(p == 1),
                        )
                nc.vector.tensor_copy(yo[:, jj, :], fp)
                if jj == 1:
                    nc.sync.dma_start(
                        out=y_d[:][
                            qb * QB + g * 256 : qb * QB + (g + 1) * 256, :
                        ].rearrange("(r p) n -> p r n", p=128),
                        in_=yo_box.pop("yo"),
                    )

            def emit_attention(
                qb, extras, ut_pairs, deferred, extra_ok=lambda p, st: True,
                last=False, prefetch=(),
            ):
                """Two pair-phases; scores of a pair are adjacent (row groups
                0-63/64-127 run concurrently on the PE array); U matmuls lag
                scores by LAG tiles. Extra units (k/v/q projection groups,
                previous q-block's fc halves) are drip-fed one per step so
                the PE never idles long enough to re-throttle; on qb1-pair0
                they start at st>=4 so the deferred normalization DMA chain
                lands first. The off-chip normalization of pair p is emitted
                at the start of pair p+1; the final pair uses the on-chip
                fast path instead."""
                ups, pts = {}, {}
                for pair in range(2):
                    h0, h1 = 2 * pair, 2 * pair + 1
                    ups[h0] = ps_a.tile([128, QB], f32, tag="acc", name="upA")
                    ups[h1] = ps_a.tile([128, QB], f32, tag="acc", name="upB")
                    ext = extras.get(pair, [])
                    for st in range(NKT + LAG):
                        if st == 0 and deferred:
                            deferred.pop(0)()
                        if pair == 1 and st < 4 and prefetch:
                            prefetch.pop(0)()
                        if ext and extra_ok(pair, st):
                            ext.pop(0)()
                        if st < NKT:
                            scores(h0, st, qb, pts)
                            scores(h1, st, qb, pts)
                        if st >= LAG:
                            umm(h0, st - LAG, ups, pts)
                            umm(h1, st - LAG, ups, pts)
                    state, gp = {}, (qb, pair)
                    norm_pair_evict(pair, ups, state)
                    if last and pair == 1:
                        norm_pair_finish(gp, state, ut_pairs)
                    else:
                        deferred.append(
                            lambda gp=gp, state=state: norm_pair_finish(gp, state, ut_pairs)
                        )
                for ext in extras.values():
                    for t in ext:
                        t()

            # ---------- main emission ----------
            # prologue: exactly what attention qb0-pair0 needs to start; the
            # mask/wv/wf loads queue behind it so they don't delay the first
            # matmuls
            proj_qk_unit("k", 0)
            proj_qk_unit("q", 0)
            proj_qk_unit("q", 1)
            mask_load(0, 0)
            t = consts.tile([128, KC, HD], f16, tag="wv", name="wv")
            nc.sync.dma_start(out=t, in_=wv_d[:])
            w_sb["v"] = t
            for grp in range(1, 4):
                mask_load(0, grp)
            wf_sb = consts.tile([128, 2, D], f16, tag="wf")
            nc.sync.dma_start(out=wf_sb, in_=wf_d[:])

            ut_pairs, deferred = {}, []
            # software-pipelined projection extras: each unit's x DMA issues
            # two slots before its matmuls so the PE never waits on HBM
            units = [("v", 0), ("k", 1), ("v", 1), ("k", 2), ("v", 2), ("k", 3), ("v", 3)]
            uboxes = [{} for _ in units + [None, None]]

            def u_dma(i, proj, g):
                uboxes[i]["x"] = proj_dma(proj, g)

            def u_comp(i, proj, g):
                if proj == "v":
                    proj_v_unit(g, uboxes[i].pop("x"))
                else:
                    proj_qk_unit(proj, g, xt2=uboxes[i].pop("x"))

            pipe0 = []
            for i, (proj, g) in enumerate(units):
                pipe0.append(lambda i=i, proj=proj, g=g: u_dma(i, proj, g))
                if i >= 1:
                    pj, gj = units[i - 1]
                    pipe0.append(lambda i=i - 1, proj=pj, g=gj: u_comp(i, proj, g))
            pj, gj = units[-1]
            pipe0.append(lambda i=len(units) - 1, proj=pj, g=gj: u_comp(i, proj, g))
            pipe0.append(lambda: u_dma(7, "q", 2))
            pipe0.append(lambda: u_dma(8, "q", 3))
            pipe1 = [
                lambda: u_comp(7, "q", 2),
                lambda: u_comp(8, "q", 3),
            ]
            emit_attention(
                0, {0: pipe0, 1: pipe1}, ut_pairs, deferred,
                prefetch=[lambda grp=grp: mask_load(1, grp) for grp in range(4)],
            )
            uts0 = lambda p: ut_pairs[(0, p)]
            boxes0 = [{} for _ in range(4)]
            fc_halves = [
                lambda g=g, jj=jj: fc_half(0, g, jj, boxes0[g], uts0)
                for g in range(4)
                for jj in range(2)
            ]
            emit_attention(
                1, {0: fc_halves[:4], 1: fc_halves[4:]}, ut_pairs, deferred,
                extra_ok=lambda p, st: st % 2 == 0 and (p == 1 or st >= 2),
                last=True,
            )
            for t_ in deferred:
                t_()
            uts1 = lambda p: ut_pairs[(1, p)]
            boxes1 = [{} for _ in range(4)]
            for g in range(4):
                for jj in range(2):
                    fc_half(1, g, jj, boxes1[g], uts1)

    nc.compile()
    return nc


def get_nc():
    if "nc" not in _CACHE:
        _CACHE["nc"] = _build()
    return _CACHE["nc"]


def make_in_maps(q, k, v, mask, wq, bq, wk, wv, wf):
    q = np.asarray(q, np.float32)
    k = np.asarray(k, np.float32)
    v = np.asarray(v, np.float32)
    def tile_x(x):
        # [S, D] -> x^T tiled as [128, S/256, KC, 256]:
        # element (c*128+p, j*256+s) -> [p, j, c, s]
        xt = x.T.astype(np.float16).reshape(KC, 128, S // 256, 256)
        return np.ascontiguousarray(xt.transpose(1, 2, 0, 3))

    xqT = [tile_x(q[b]) for b in range(B)]
    xkT = [tile_x(k[b]) for b in range(B)]
    xvT = [tile_x(v[b]) for b in range(B)]
    def tile_mask(m):
        # mask^T [sk, sq] -> [128, NQB, NKT, QB]: (st*128+p, qb*QB+s) -> [p, qb, st, s]
        mt = m.T.astype(np.float16).reshape(NKT, 128, NQB, QB)
        return np.ascontiguousarray(mt.transpose(1, 2, 0, 3))

    mkT = [tile_mask(np.asarray(mask[b])) for b in range(B)]
    wq = np.asarray(wq, np.float16)
    wk = np.asarray(wk, np.float16)
    wv = np.asarray(wv, np.float16)
    wf = np.asarray(wf, np.float16)
    bq = np.asarray(bq, np.float32)
    in_maps = []
    for c in range(NCORES):
        b, hg = c // HPC, c % HPC
        cols = slice(hg * HD, (hg + 1) * HD)
        in_maps.append(
            {
                "xqT": xqT[b],
                "xkT": xkT[b],
                "xvT": xvT[b],
                "wq": np.ascontiguousarray(
                    wq[:, cols].reshape(KC, 128, HD).transpose(1, 0, 2)
                ),
                "wk": np.ascontiguousarray(
                    wk[:, cols].reshape(KC, 128, HD).transpose(1, 0, 2)
                ),
                "wv": np.ascontiguousarray(
                    wv[:, cols].reshape(KC, 128, HD).transpose(1, 0, 2)
                ),
                "bq": np.ascontiguousarray(bq[cols].reshape(2, 128).T),
                # head-pair stacked fc weights: [128 rows of pair p, p, D]
                "wf": np.ascontiguousarray(
                    wf[cols, :].reshape(2, 128, D).transpose(1, 0, 2)
                ),
                "maskT": mkT[b],
            }
        )
    return in_maps


LAST_RESULTS = None


def kernel(q, k, v, mask, wq, bq, wk, wv, bv, wf, bf, **trace_kwargs):
    from concourse.bass_utils import run_bass_kernel_spmd

    global LAST_RESULTS
    nc = get_nc()
    in_maps = make_in_maps(q, k, v, mask, wq, bq, wk, wv, wf)
    res = run_bass_kernel_spmd(
        nc, in_maps, core_ids=list(range(NCORES)), **trace_kwargs
    )
    LAST_RESULTS = res
    out = np.zeros((B, S, D), np.float64)
    for c in range(NCORES):
        out[c // HPC] += res.results[c]["y"].astype(np.float64)
    extra = (
        np.asarray(bv, np.float64) @ np.asarray(wf, np.float64)
        + np.asarray(bf, np.float64)
    )
    out += extra[None, None, :]
    return out.astype(np.float32)


# revision 44
# speedup vs baseline: 1.0079x; 1.0079x over previous
"""Trainium2 Bass kernel: decoder multi-head attention (B=2, S=2048, D=1024, 16 heads).

Sharding: 8 cores = 2 batches x 4 head-groups (4 heads / 256 dims per core).
Per core (batch b, head group hg), all in transposed layouts:
  Q^T = (wq_c)^T @ xq[b]^T + bq_c      [256, 2048]
  K^T = (wk_c)^T @ xk[b]^T             [256, 2048]
  V   = xv[b] @ wv_c                   [2048, 256] token-major (no bias)
  per head h: scores^T[sk,sq] = K_h^T.T @ Q_h^T            (K=64, head pairs
              run concurrently on PE row groups 0-63/64-127)
              P^T = exp(scores^T/8) * mask^T               (fp16)
              [U^T; rowsum] = [V_h | 1].T @ P^T            (ones col -> rowsum)
  per pair p: UT2_p[128,sq] = [U_h0; U_h1] * (1/rowsum)    (recip_approx_fast
              on PSUM row 64 + gpsimd partition_broadcast; no DRAM roundtrip)
  y_partial = sum_p UT2_p.T @ wf2_p                        [2048, 1024]
              (head-pair stacked: contraction 128)
Host: out[b] = sum_hg y_partial + bv @ wf + bf
(v bias folded out: attention rows sum to 1, so attn@(V+bv) = attn@V + bv.)

Scheduling: x tiles DMA'd once per token group (shared by both w row tiles);
all mask DMAs issued up front on the gpsimd queue; PSUM evictions on DVE
(ACT does exp only); q-proj for the second q-block and fc for the previous
q-block ride in the attention pair-boundary bubbles.
"""

import sys

if "/opt/trn_rl_repo" not in sys.path:
    sys.path.insert(0, "/opt/trn_rl_repo")

import numpy as np

B, S, D = 2, 2048, 1024
NH, DK = 16, 64
NCORES = 8
HPC = 4            # heads per core
HD = HPC * DK      # 256 head dims per core
QB = 1024          # q-block (free dim of scores^T tiles)
NQB = S // QB      # 2
NKT = S // 128     # 16 sk tiles
KC = D // 128      # 8 contraction chunks for projections
LAG = 3            # umm trails scores by LAG sk-tiles

_CACHE = {}


def _build():
    import concourse.mybir as mybir
    import concourse.tile as tile
    from concourse import bacc

    f32 = mybir.dt.float32
    f16 = mybir.dt.float16
    EXP = mybir.ActivationFunctionType.Exp

    nc = bacc.Bacc(
        "TRN2",
        target_bir_lowering=False,
        debug=False,
        enable_asserts=False,
        num_devices=NCORES,
    )

    xq_d = nc.dram_tensor("xqT", [128, S // 256, KC, 256], f16, kind="ExternalInput")
    xk_d = nc.dram_tensor("xkT", [128, S // 256, KC, 256], f16, kind="ExternalInput")
    xv_d = nc.dram_tensor("xvT", [128, S // 256, KC, 256], f16, kind="ExternalInput")
    wq_d = nc.dram_tensor("wq", [128, KC, HD], f16, kind="ExternalInput")
    wk_d = nc.dram_tensor("wk", [128, KC, HD], f16, kind="ExternalInput")
    wv_d = nc.dram_tensor("wv", [128, KC, HD], f16, kind="ExternalInput")
    bq_d = nc.dram_tensor("bq", [128, 2], f32, kind="ExternalInput")
    wf_d = nc.dram_tensor("wf", [128, 2, D], f16, kind="ExternalInput")
    mk_d = nc.dram_tensor("maskT", [128, NQB, NKT, QB], f16, kind="ExternalInput")
    y_d = nc.dram_tensor("y", [S, D], f16, kind="ExternalOutput")

    with tile.TileContext(nc) as tc:
        with (
            tc.tile_pool(name="consts", bufs=1) as consts,
            tc.tile_pool(name="qk", bufs=1) as qkp,
            tc.tile_pool(name="maskc", bufs=5) as maskc,
            tc.tile_pool(name="xs", bufs=3) as xsp,
            tc.tile_pool(name="exp", bufs=5) as expp,
            tc.tile_pool(name="pt", bufs=10) as ptp,
            tc.tile_pool(name="usb", bufs=2) as usbp,
            tc.tile_pool(name="rs", bufs=2) as rsp,
            tc.tile_pool(name="bc", bufs=2) as bcp,
            tc.tile_pool(name="ut", bufs=4) as utp,
            tc.tile_pool(name="yo", bufs=2) as yop,
            tc.tile_pool(name="ps_s", bufs=2, space="PSUM") as ps_s,
            tc.tile_pool(name="ps_a", bufs=2, space="PSUM") as ps_a,
        ):
            # ---- constants (k/q first: the prologue needs only those) ----
            w_sb = {}
            for name, dram in (("k", wk_d), ("q", wq_d)):
                t = consts.tile([128, KC, HD], f16, tag=f"w{name}", name=f"w{name}")
                nc.sync.dma_start(out=t, in_=dram[:])
                w_sb[name] = t
            bq_sb = consts.tile([128, 2], f32, tag="bq")
            nc.sync.dma_start(out=bq_sb, in_=bq_d[:])

            # ---- mask tiles: qb0 right after the prologue; qb1 during
            # qb0-pair1 (issued on the gpsimd queue) ----
            mtiles = {}

            def mask_load(qb, grp):
                mg = maskc.tile([128, 4, QB], f16, tag="mask", name=f"m{qb}_{grp}")
                nc.gpsimd.dma_start(
                    out=mg, in_=mk_d[:][:, qb, 4 * grp : 4 * grp + 4, :]
                )
                mtiles[(qb, grp)] = mg

            # persistent activations
            QT = [qkp.tile([128, S], f16, tag=f"qt{m}", name=f"qt{m}") for m in range(2)]
            KT = [qkp.tile([128, S], f16, tag=f"kt{m}", name=f"kt{m}") for m in range(2)]
            V = [qkp.tile([128, HPC * 65], f16, tag=f"v{st}", name=f"v{st}") for st in range(NKT)]

            # ---------- emission units ----------
            def proj_dma(proj, g):
                """Issue one token group's 1MB x DMA; returns the tile."""
                src_d = {"q": xq_d, "k": xk_d, "v": xv_d}[proj]
                xt2 = xsp.tile([128, 2, KC, 256], f16, tag="xs", name="xt2")
                nc.sync.dma_start(out=xt2, in_=src_d[:][:, 2 * g : 2 * g + 2, :, :])
                return xt2

            def proj_qk_unit(proj, g, ms=(0, 1), xt2=None):
                """One token group (512 tokens) of q/k projection: one 1MB x
                DMA shared by the requested 128-row weight tiles; evictions
                on DVE."""
                if xt2 is None:
                    xt2 = proj_dma(proj, g)
                for m in ms:
                    for jj in range(2):
                        j = 2 * g + jj
                        xt = xt2[:, jj, :, :]
                        ps = ps_s.tile([128, QB], f32, tag="sc", name="ps")
                        for kc in range(KC):
                            nc.tensor.matmul(
                                ps[:, :256],
                                lhsT=w_sb[proj][:, kc, m * 128 : (m + 1) * 128],
                                rhs=xt[:, kc, :],
                                start=(kc == 0),
                                stop=(kc == KC - 1),
                            )
                        dst = (QT if proj == "q" else KT)[m][:, j * 256 : (j + 1) * 256]
                        if proj == "q":
                            nc.vector.tensor_scalar_add(dst, ps[:, :256], bq_sb[:, m : m + 1])
                        else:
                            nc.vector.tensor_copy(dst, ps[:, :256])

            def proj_v_unit(g, xt2=None):
                """One token group of v projection (4 sk tiles, token-major)."""
                if xt2 is None:
                    xt2 = proj_dma("v", g)
                for jj in range(2):
                    for sub in range(2):
                        st = (2 * g + jj) * 2 + sub
                        ps = ps_s.tile([128, QB], f32, tag="sc", name="ps")
                        for kc in range(KC):
                            nc.tensor.matmul(
                                ps[:, :HD],
                                lhsT=xt2[:, jj, kc, sub * 128 : (sub + 1) * 128],
                                rhs=w_sb["v"][:, kc, :],
                                start=(kc == 0),
                                stop=(kc == KC - 1),
                            )
                        vt = V[st]
                        vt3 = vt.rearrange("p (h c) -> p h c", h=HPC)
                        nc.gpsimd.memset(vt3[:, :, 64:65], 1.0)
                        nc.vector.tensor_copy(
                            vt3[:, :, 0:64],
                            ps[:, :HD].rearrange("p (h c) -> p h c", h=HPC),
                        )

            def scores(h, st, qb, pts):
                c, r = h // 2, 64 * (h % 2)
                ps = ps_s.tile([128, QB], f32, tag="sc", name="sc")
                for half in range(2):
                    nc.tensor.matmul(
                        ps[:, half * 512 : (half + 1) * 512],
                        lhsT=KT[c][r : r + 64, st * 128 : (st + 1) * 128],
                        rhs=QT[c][
                            r : r + 64,
                            qb * QB + half * 512 : qb * QB + (half + 1) * 512,
                        ],
                        start=True,
                        stop=True,
                    )
                et = expp.tile([128, QB], f16, tag="exp", name="et")
                nc.scalar.activation(et, ps, EXP, scale=0.125)
                pt = ptp.tile([128, QB], f16, tag="pt", name="pt")
                nc.vector.tensor_mul(pt, et, mtiles[(qb, st // 4)][:, st % 4, :])
                pts[(h, st)] = pt

            def umm(h, st, ups, pts):
                pt = pts.pop((h, st))
                up = ups[h]
                for half in range(2):
                    nc.tensor.matmul(
                        up[0:65, half * 512 : (half + 1) * 512],
                        lhsT=V[st][:, 65 * h : 65 * h + 65],
                        rhs=pt[:, half * 512 : (half + 1) * 512],
                        start=(st == 0),
                        stop=(st == NKT - 1),
                    )

            def norm_pair_evict(pair, ups, state):
                """Pair-end PSUM readout (frees the accumulators fast): U of
                each head into its own base-0 [64, QB] tile, rowsums into
                base-0 [1, QB] tiles. The reciprocal/broadcast/multiply is
                deferred (norm_pair_finish) to the next pair so it never
                blocks the DVE queue."""
                h0, h1 = 2 * pair, 2 * pair + 1
                up0, up1 = ups.pop(h0), ups.pop(h1)
                usb_lo = usbp.tile([64, QB], f32, tag="usb_lo", name="usb_lo")
                nc.vector.tensor_copy(usb_lo, up0[0:64, :])
                usb_hi = usbp.tile([64, QB], f32, tag="usb_hi", name="usb_hi")
                nc.vector.tensor_copy(usb_hi, up1[0:64, :])
                rs_a = rsp.tile([1, QB], f32, tag="rs_a", name="rs_a")
                nc.vector.tensor_copy(rs_a, up0[64:65, :])
                rs_b = rsp.tile([1, QB], f32, tag="rs_b", name="rs_b")
                nc.vector.tensor_copy(rs_b, up1[64:65, :])
                state["t"] = (usb_lo, usb_hi, rs_a, rs_b)

            def norm_pair_finish(gp, state, ut_pairs):
                """All on-chip: in-place approx reciprocal (base-0 custom op),
                gpsimd partition_broadcast into base-0 [64, QB] tiles, two DVE
                multiplies (both inputs base-0; only the output base differs)."""
                usb_lo, usb_hi, rs_a, rs_b = state.pop("t")
                nc.vector.reciprocal_approx_fast(out=rs_a, in_=rs_a)
                nc.vector.reciprocal_approx_fast(out=rs_b, in_=rs_b)
                bc0 = bcp.tile([64, QB], f32, tag="bc0", name="bc0")
                nc.gpsimd.partition_broadcast(bc0, rs_a)
                bc1 = bcp.tile([64, QB], f32, tag="bc1", name="bc1")
                nc.gpsimd.partition_broadcast(bc1, rs_b)
                ut2 = utp.tile([128, QB], f16, tag="ut", name="ut2")
                nc.vector.tensor_mul(ut2[0:64, :], usb_lo, bc0)
                nc.vector.tensor_mul(ut2[64:128, :], usb_hi, bc1)
                ut_pairs[gp] = ut2

            def fc_half(qb, g, jj, yo_box, ut_get):
                # one 128-row fc tile; on jj==1 ships the [256, D] yo tile
                if jj == 0:
                    yo_box["yo"] = yop.tile([128, 2, D], f16, tag="yo", name="yo")
                yo = yo_box["yo"]
                j = 2 * g + jj
                fp = ps_s.tile([128, QB], f32, tag="sc", name="fp")
                for half in range(2):
                    for p in range(2):
                        nc.tensor.matmul(
                            fp[:, half * 512 : (half + 1) * 512],
                            lhsT=ut_get(p)[:, j * 128 : (j + 1) * 128],
                            rhs=wf_sb[:, p, half * 512 : (half + 1) * 512],
                            start=(p == 0),
                            stop=(p == 1),
                        )
                nc.vector.tensor_copy(yo[:, jj, :], fp)
                if jj == 1:
                    nc.sync.dma_start(
                        out=y_d[:][
                            qb * QB + g * 256 : qb * QB + (g + 1) * 256, :
                        ].rearrange("(r p) n -> p r n", p=128),
                        in_=yo_box.pop("yo"),
                    )

            def emit_attention(
                qb, extras, ut_pairs, deferred, extra_ok=lambda p, st: True,
                last=False, prefetch=(),
            ):
                """Two pair-phases; scores of a pair are adjacent (row groups
                0-63/64-127 run concurrently on the PE array); U matmuls lag
                scores by LAG tiles. Extra units (k/v/q projection groups,
                previous q-block's fc halves) are drip-fed one per step so
                the PE never idles long enough to re-throttle; on qb1-pair0
                they start at st>=4 so the deferred normalization DMA chain
                lands first. The off-chip normalization of pair p is emitted
                at the start of pair p+1; the final pair uses the on-chip
                fast path instead."""
                ups, pts = {}, {}
                for pair in range(2):
                    h0, h1 = 2 * pair, 2 * pair + 1
                    ups[h0] = ps_a.tile([128, QB], f32, tag="acc", name="upA")
                    ups[h1] = ps_a.tile([128, QB], f32, tag="acc", name="upB")
                    ext = extras.get(pair, [])
                    for st in range(NKT + LAG):
                        if st == 0 and deferred:
                            deferred.pop(0)()
                        if pair == 1 and st < 4 and prefetch:
                            prefetch.pop(0)()
                        if ext and extra_ok(pair, st):
                            ext.pop(0)()
                        if st < NKT:
                            scores(h0, st, qb, pts)
                            scores(h1, st, qb, pts)
                        if st >= LAG:
                            umm(h0, st - LAG, ups, pts)
                            umm(h1, st - LAG, ups, pts)
                    state, gp = {}, (qb, pair)
                    norm_pair_evict(pair, ups, state)
                    if last and pair == 1:
                        norm_pair_finish(gp, state, ut_pairs)
                    else:
                        deferred.append(
                            lambda gp=gp, state=state: norm_pair_finish(gp, state, ut_pairs)
                        )
                for ext in extras.values():
                    for t in ext:
                        t()

            # ---------- main emission ----------
            # prologue: exactly what attention qb0-pair0 needs to start; the
            # mask/wv/wf loads queue behind it so they don't delay the first
            # matmuls
            proj_qk_unit("k", 0)
            proj_qk_unit("q", 0)
            proj_qk_unit("q", 1)
            mask_load(0, 0)
            t = consts.tile([128, KC, HD], f16, tag="wv", name="wv")
            nc.sync.dma_start(out=t, in_=wv_d[:])
            w_sb["v"] = t
            for grp in range(1, 4):
                mask_load(0, grp)
            wf_sb = consts.tile([128, 2, D], f16, tag="wf")
            nc.sync.dma_start(out=wf_sb, in_=wf_d[:])

            ut_pairs, deferred = {}, []
            # projection extras run as early as dependencies allow; only the
            # pair-boundary fillers (q-block-1 projections) get their x DMAs
            # issued ahead so their matmuls never wait on HBM
            qbox = [{}, {}]
            vbox = {}
            vbox["x"] = proj_dma("v", 0)
            pipe0 = [
                lambda: proj_v_unit(0, vbox.pop("x")),
                lambda: proj_qk_unit("k", 1),
                lambda: proj_v_unit(1),
                lambda: proj_qk_unit("k", 2),
                lambda: proj_v_unit(2),
                lambda: proj_qk_unit("k", 3),
                lambda: proj_v_unit(3),
                lambda: qbox[0].update(x=proj_dma("q", 2)),
                lambda: qbox[1].update(x=proj_dma("q", 3)),
            ]
            pipe1 = [
                lambda: proj_qk_unit("q", 2, xt2=qbox[0].pop("x")),
                lambda: proj_qk_unit("q", 3, xt2=qbox[1].pop("x")),
            ]
            emit_attention(
                0, {0: pipe0, 1: pipe1}, ut_pairs, deferred,
                prefetch=[lambda grp=grp: mask_load(1, grp) for grp in range(4)],
            )
            uts0 = lambda p: ut_pairs[(0, p)]
            boxes0 = [{} for _ in range(4)]
            fc_halves = [
                lambda g=g, jj=jj: fc_half(0, g, jj, boxes0[g], uts0)
                for g in range(4)
                for jj in range(2)
            ]
            emit_attention(
                1, {0: fc_halves[:5], 1: fc_halves[5:]}, ut_pairs, deferred,
                extra_ok=lambda p, st: p == 1 or st >= 4,
                last=True,
            )
            for t_ in deferred:
                t_()
            uts1 = lambda p: ut_pairs[(1, p)]
            boxes1 = [{} for _ in range(4)]
            for g in range(4):
                for jj in range(2):
                    fc_half(1, g, jj, boxes1[g], uts1)

    nc.compile()
    return nc


def get_nc():
    if "nc" not in _CACHE:
        _CACHE["nc"] = _build()
    return _CACHE["nc"]


def make_in_maps(q, k, v, mask, wq, bq, wk, wv, wf):
    q = np.asarray(q, np.float32)
    k = np.asarray(k, np.float32)
    v = np.asarray(v, np.float32)
    def tile_x(x):
        # [S, D] -> x^T tiled as [128, S/256, KC, 256]:
        # element (c*128+p, j*256+s) -> [p, j, c, s]
        xt = x.T.astype(np.float16).reshape(KC, 128, S // 256, 256)
        return np.ascontiguousarray(xt.transpose(1, 2, 0, 3))

    xqT = [tile_x(q[b]) for b in range(B)]
    xkT = [tile_x(k[b]) for b in range(B)]
    xvT = [tile_x(v[b]) for b in range(B)]
    def tile_mask(m):
        # mask^T [sk, sq] -> [128, NQB, NKT, QB]: (st*128+p, qb*QB+s) -> [p, qb, st, s]
        mt = m.T.astype(np.float16).reshape(NKT, 128, NQB, QB)
        return np.ascontiguousarray(mt.transpose(1, 2, 0, 3))

    mkT = [tile_mask(np.asarray(mask[b])) for b in range(B)]
    wq = np.asarray(wq, np.float16)
    wk = np.asarray(wk, np.float16)
    wv = np.asarray(wv, np.float16)
    wf = np.asarray(wf, np.float16)
    bq = np.asarray(bq, np.float32)
    in_maps = []
    for c in range(NCORES):
        b, hg = c // HPC, c % HPC
        cols = slice(hg * HD, (hg + 1) * HD)
        in_maps.append(
            {
                "xqT": xqT[b],
                "xkT": xkT[b],
                "xvT": xvT[b],
                "wq": np.ascontiguousarray(
                    wq[:, cols].reshape(KC, 128, HD).transpose(1, 0, 2)
                ),
                "wk": np.ascontiguousarray(
                    wk[:, cols].reshape(KC, 128, HD).transpose(1, 0, 2)
                ),
                "wv": np.ascontiguousarray(
                    wv[:, cols].reshape(KC, 128, HD).transpose(1, 0, 2)
                ),
                "bq": np.ascontiguousarray(bq[cols].reshape(2, 128).T),
                # head-pair stacked fc weights: [128 rows of pair p, p, D]
                "wf": np.ascontiguousarray(
                    wf[cols, :].reshape(2, 128, D).transpose(1, 0, 2)
                ),
                "maskT": mkT[b],
            }
        )
    return in_maps


LAST_RESULTS = None


def kernel(q, k, v, mask, wq, bq, wk, wv, bv, wf, bf, **trace_kwargs):
    from concourse.bass_utils import run_bass_kernel_spmd

    global LAST_RESULTS
    nc = get_nc()
    in_maps = make_in_maps(q, k, v, mask, wq, bq, wk, wv, wf)
    res = run_bass_kernel_spmd(
        nc, in_maps, core_ids=list(range(NCORES)), **trace_kwargs
    )
    LAST_RESULTS = res
    out = np.zeros((B, S, D), np.float64)
    for c in range(NCORES):
        out[c // HPC] += res.results[c]["y"].astype(np.float64)
    extra = (
        np.asarray(bv, np.float64) @ np.asarray(wf, np.float64)
        + np.asarray(bf, np.float64)
    )
    out += extra[None, None, :]
    return out.astype(np.float32)


# revision 45
# speedup vs baseline: 1.1689x; 1.1598x over previous
"""Trainium2 Bass kernel: decoder multi-head attention (B=2, S=2048, D=1024, 16 heads).

Sharding: 8 cores = 2 batches x 4 head-groups (4 heads / 256 dims per core).
Per core (batch b, head group hg), all in transposed layouts:
  Q^T = (wq_c)^T @ xq[b]^T + bq_c      [256, 2048]
  K^T = (wk_c)^T @ xk[b]^T             [256, 2048]
  V   = xv[b] @ wv_c                   [2048, 256] token-major (no bias)
  per head h: scores^T[sk,sq] = K_h^T.T @ Q_h^T            (K=64, head pairs
              run concurrently on PE row groups 0-63/64-127)
              P^T = exp(scores^T/8) * mask^T               (fp16)
              [U^T; rowsum] = [V_h | 1].T @ P^T            (ones col -> rowsum)
  per pair p: UT2_p[128,sq] = [U_h0; U_h1] * (1/rowsum)    (recip_approx_fast
              on PSUM row 64 + gpsimd partition_broadcast; no DRAM roundtrip)
  y_partial = sum_p UT2_p.T @ wf2_p                        [2048, 1024]
              (head-pair stacked: contraction 128)
Host: out[b] = sum_hg y_partial + bv @ wf + bf
(v bias folded out: attention rows sum to 1, so attn@(V+bv) = attn@V + bv.)

Scheduling: x tiles DMA'd once per token group (shared by both w row tiles);
all mask DMAs issued up front on the gpsimd queue; PSUM evictions on DVE
(ACT does exp only); q-proj for the second q-block and fc for the previous
q-block ride in the attention pair-boundary bubbles.
"""

import sys

if "/opt/trn_rl_repo" not in sys.path:
    sys.path.insert(0, "/opt/trn_rl_repo")

import numpy as np

B, S, D = 2, 2048, 1024
NH, DK = 16, 64
NCORES = 8
HPC = 4            # heads per core
HD = HPC * DK      # 256 head dims per core
QB = 1024          # q-block (free dim of scores^T tiles)
NQB = S // QB      # 2
NKT = S // 128     # 16 sk tiles
KC = D // 128      # 8 contraction chunks for projections
LAG = 3            # umm trails scores by LAG sk-tiles

_CACHE = {}


def _build():
    import concourse.mybir as mybir
    import concourse.tile as tile
    from concourse import bacc

    f32 = mybir.dt.float32
    f16 = mybir.dt.float16
    EXP = mybir.ActivationFunctionType.Exp

    nc = bacc.Bacc(
        "TRN2",
        target_bir_lowering=False,
        debug=False,
        enable_asserts=False,
        num_devices=NCORES,
    )

    xq_d = nc.dram_tensor("xqT", [128, S // 256, KC, 256], f16, kind="ExternalInput")
    xk_d = nc.dram_tensor("xkT", [128, S // 256, KC, 256], f16, kind="ExternalInput")
    xv_d = nc.dram_tensor("xvT", [128, S // 256, KC, 256], f16, kind="ExternalInput")
    wq_d = nc.dram_tensor("wq", [128, KC, HD], f16, kind="ExternalInput")
    wk_d = nc.dram_tensor("wk", [128, KC, HD], f16, kind="ExternalInput")
    wv_d = nc.dram_tensor("wv", [128, KC, HD], f16, kind="ExternalInput")
    bq_d = nc.dram_tensor("bq", [128, 2], f32, kind="ExternalInput")
    wf_d = nc.dram_tensor("wf", [128, 2, D], f16, kind="ExternalInput")
    mk_d = nc.dram_tensor("maskT", [128, NQB, NKT, QB], f16, kind="ExternalInput")
    y_d = nc.dram_tensor("y", [S, D], f16, kind="ExternalOutput")

    with tile.TileContext(nc) as tc:
        with (
            tc.tile_pool(name="consts", bufs=1) as consts,
            tc.tile_pool(name="qk", bufs=1) as qkp,
            tc.tile_pool(name="maskc", bufs=5) as maskc,
            tc.tile_pool(name="xs", bufs=2) as xsp,
            tc.tile_pool(name="exp", bufs=5) as expp,
            tc.tile_pool(name="pt", bufs=10) as ptp,
            tc.tile_pool(name="usb", bufs=2) as usbp,
            tc.tile_pool(name="rs", bufs=2) as rsp,
            tc.tile_pool(name="bc", bufs=2) as bcp,
            tc.tile_pool(name="ut", bufs=4) as utp,
            tc.tile_pool(name="yo", bufs=2) as yop,
            tc.tile_pool(name="ps_s", bufs=2, space="PSUM") as ps_s,
            tc.tile_pool(name="ps_a", bufs=2, space="PSUM") as ps_a,
        ):
            # ---- constants (k/q first: the prologue needs only those) ----
            w_sb = {}
            for name, dram in (("k", wk_d), ("q", wq_d)):
                t = consts.tile([128, KC, HD], f16, tag=f"w{name}", name=f"w{name}")
                nc.sync.dma_start(out=t, in_=dram[:])
                w_sb[name] = t
            bq_sb = consts.tile([128, 2], f32, tag="bq")
            nc.sync.dma_start(out=bq_sb, in_=bq_d[:])

            # ---- mask tiles: qb0 right after the prologue; qb1 during
            # qb0-pair1 (issued on the gpsimd queue) ----
            mtiles = {}

            def mask_load(qb, grp):
                mg = maskc.tile([128, 4, QB], f16, tag="mask", name=f"m{qb}_{grp}")
                nc.gpsimd.dma_start(
                    out=mg, in_=mk_d[:][:, qb, 4 * grp : 4 * grp + 4, :]
                )
                mtiles[(qb, grp)] = mg

            # persistent activations
            QT = [qkp.tile([128, S], f16, tag=f"qt{m}", name=f"qt{m}") for m in range(2)]
            KT = [qkp.tile([128, S], f16, tag=f"kt{m}", name=f"kt{m}") for m in range(2)]
            V = [qkp.tile([128, HPC * 65], f16, tag=f"v{st}", name=f"v{st}") for st in range(NKT)]

            # ---------- emission units ----------
            def proj_dma(proj, g):
                """Issue one token group's 1MB x DMA; returns the tile."""
                src_d = {"q": xq_d, "k": xk_d, "v": xv_d}[proj]
                xt2 = xsp.tile([128, 2, KC, 256], f16, tag="xs", name="xt2")
                nc.sync.dma_start(out=xt2, in_=src_d[:][:, 2 * g : 2 * g + 2, :, :])
                return xt2

            def proj_qk_unit(proj, g, ms=(0, 1), xt2=None):
                """One token group (512 tokens) of q/k projection: one 1MB x
                DMA shared by the requested 128-row weight tiles; evictions
                on DVE."""
                if xt2 is None:
                    xt2 = proj_dma(proj, g)
                for m in ms:
                    for jj in range(2):
                        j = 2 * g + jj
                        xt = xt2[:, jj, :, :]
                        ps = ps_s.tile([128, QB], f32, tag="sc", name="ps")
                        for kc in range(KC):
                            nc.tensor.matmul(
                                ps[:, :256],
                                lhsT=w_sb[proj][:, kc, m * 128 : (m + 1) * 128],
                                rhs=xt[:, kc, :],
                                start=(kc == 0),
                                stop=(kc == KC - 1),
                            )
                        dst = (QT if proj == "q" else KT)[m][:, j * 256 : (j + 1) * 256]
                        if proj == "q":
                            nc.vector.tensor_scalar_add(dst, ps[:, :256], bq_sb[:, m : m + 1])
                        else:
                            nc.vector.tensor_copy(dst, ps[:, :256])

            def proj_v_unit(g, xt2=None):
                """One token group of v projection (4 sk tiles, token-major)."""
                if xt2 is None:
                    xt2 = proj_dma("v", g)
                for jj in range(2):
                    for sub in range(2):
                        st = (2 * g + jj) * 2 + sub
                        ps = ps_s.tile([128, QB], f32, tag="sc", name="ps")
                        for kc in range(KC):
                            nc.tensor.matmul(
                                ps[:, :HD],
                                lhsT=xt2[:, jj, kc, sub * 128 : (sub + 1) * 128],
                                rhs=w_sb["v"][:, kc, :],
                                start=(kc == 0),
                                stop=(kc == KC - 1),
                            )
                        vt = V[st]
                        vt3 = vt.rearrange("p (h c) -> p h c", h=HPC)
                        nc.gpsimd.memset(vt3[:, :, 64:65], 1.0)
                        nc.vector.tensor_copy(
                            vt3[:, :, 0:64],
                            ps[:, :HD].rearrange("p (h c) -> p h c", h=HPC),
                        )

            def scores(h, st, qb, pts):
                c, r = h // 2, 64 * (h % 2)
                ps = ps_s.tile([128, QB], f32, tag="sc", name="sc")
                for half in range(2):
                    nc.tensor.matmul(
                        ps[:, half * 512 : (half + 1) * 512],
                        lhsT=KT[c][r : r + 64, st * 128 : (st + 1) * 128],
                        rhs=QT[c][
                            r : r + 64,
                            qb * QB + half * 512 : qb * QB + (half + 1) * 512,
                        ],
                        start=True,
                        stop=True,
                    )
                et = expp.tile([128, QB], f16, tag="exp", name="et")
                nc.scalar.activation(et, ps, EXP, scale=0.125)
                pt = ptp.tile([128, QB], f16, tag="pt", name="pt")
                nc.vector.tensor_mul(pt, et, mtiles[(qb, st // 4)][:, st % 4, :])
                pts[(h, st)] = pt

            def umm(h, st, ups, pts):
                pt = pts.pop((h, st))
                up = ups[h]
                for half in range(2):
                    nc.tensor.matmul(
                        up[0:65, half * 512 : (half + 1) * 512],
                        lhsT=V[st][:, 65 * h : 65 * h + 65],
                        rhs=pt[:, half * 512 : (half + 1) * 512],
                        start=(st == 0),
                        stop=(st == NKT - 1),
                    )

            def norm_pair_evict(pair, ups, state):
                """Pair-end PSUM readout (frees the accumulators fast): U of
                each head into its own base-0 [64, QB] tile, rowsums into
                base-0 [1, QB] tiles. The reciprocal/broadcast/multiply is
                deferred (norm_pair_finish) to the next pair so it never
                blocks the DVE queue."""
                h0, h1 = 2 * pair, 2 * pair + 1
                up0, up1 = ups.pop(h0), ups.pop(h1)
                usb_lo = usbp.tile([64, QB], f32, tag="usb_lo", name="usb_lo")
                nc.vector.tensor_copy(usb_lo, up0[0:64, :])
                usb_hi = usbp.tile([64, QB], f32, tag="usb_hi", name="usb_hi")
                nc.vector.tensor_copy(usb_hi, up1[0:64, :])
                rs_a = rsp.tile([1, QB], f32, tag="rs_a", name="rs_a")
                nc.vector.tensor_copy(rs_a, up0[64:65, :])
                rs_b = rsp.tile([1, QB], f32, tag="rs_b", name="rs_b")
                nc.vector.tensor_copy(rs_b, up1[64:65, :])
                state["t"] = (usb_lo, usb_hi, rs_a, rs_b)

            def norm_pair_finish(gp, state, ut_pairs):
                """All on-chip: in-place approx reciprocal (base-0 custom op),
                gpsimd partition_broadcast into base-0 [64, QB] tiles, two DVE
                multiplies (both inputs base-0; only the output base differs)."""
                usb_lo, usb_hi, rs_a, rs_b = state.pop("t")
                nc.vector.reciprocal_approx_fast(out=rs_a, in_=rs_a)
                nc.vector.reciprocal_approx_fast(out=rs_b, in_=rs_b)
                bc0 = bcp.tile([64, QB], f32, tag="bc0", name="bc0")
                nc.gpsimd.partition_broadcast(bc0, rs_a)
                bc1 = bcp.tile([64, QB], f32, tag="bc1", name="bc1")
                nc.gpsimd.partition_broadcast(bc1, rs_b)
                ut2 = utp.tile([128, QB], f16, tag="ut", name="ut2")
                nc.vector.tensor_mul(ut2[0:64, :], usb_lo, bc0)
                nc.vector.tensor_mul(ut2[64:128, :], usb_hi, bc1)
                ut_pairs[gp] = ut2

            def fc_half(qb, g, jj, yo_box, ut_get):
                # one 128-row fc tile; on jj==1 ships the [256, D] yo tile
                if jj == 0:
                    yo_box["yo"] = yop.tile([128, 2, D], f16, tag="yo", name="yo")
                yo = yo_box["yo"]
                j = 2 * g + jj
                fp = ps_s.tile([128, QB], f32, tag="sc", name="fp")
                for half in range(2):
                    for p in range(2):
                        nc.tensor.matmul(
                            fp[:, half * 512 : (half + 1) * 512],
                            lhsT=ut_get(p)[:, j * 128 : (j + 1) * 128],
                            rhs=wf_sb[:, p, half * 512 : (half + 1) * 512],
                            start=(p == 0),
                            stop=(p == 1),
                        )
                nc.vector.tensor_copy(yo[:, jj, :], fp)
                if jj == 1:
                    nc.sync.dma_start(
                        out=y_d[:][
                            qb * QB + g * 256 : qb * QB + (g + 1) * 256, :
                        ].rearrange("(r p) n -> p r n", p=128),
                        in_=yo_box.pop("yo"),
                    )

            def emit_attention(
                qb, extras, ut_pairs, deferred, extra_ok=lambda p, st: True,
                last=False, prefetch=(),
            ):
                """Two pair-phases; scores of a pair are adjacent (row groups
                0-63/64-127 run concurrently on the PE array); U matmuls lag
                scores by LAG tiles. Extra units (k/v/q projection groups,
                previous q-block's fc halves) are drip-fed one per step so
                the PE never idles long enough to re-throttle; on qb1-pair0
                they start at st>=4 so the deferred normalization DMA chain
                lands first. The off-chip normalization of pair p is emitted
                at the start of pair p+1; the final pair uses the on-chip
                fast path instead."""
                ups, pts = {}, {}
                for pair in range(2):
                    h0, h1 = 2 * pair, 2 * pair + 1
                    ups[h0] = ps_a.tile([128, QB], f32, tag="acc", name="upA")
                    ups[h1] = ps_a.tile([128, QB], f32, tag="acc", name="upB")
                    ext = extras.get(pair, [])
                    for st in range(NKT + LAG):
                        if st == 0 and deferred:
                            deferred.pop(0)()
                        if pair == 1 and st < 4 and prefetch:
                            prefetch.pop(0)()
                        if ext and extra_ok(pair, st):
                            ext.pop(0)()
                        if st < NKT:
                            scores(h0, st, qb, pts)
                            scores(h1, st, qb, pts)
                        if st >= LAG:
                            umm(h0, st - LAG, ups, pts)
                            umm(h1, st - LAG, ups, pts)
                    state, gp = {}, (qb, pair)
                    norm_pair_evict(pair, ups, state)
                    if last and pair == 1:
                        norm_pair_finish(gp, state, ut_pairs)
                    else:
                        deferred.append(
                            lambda gp=gp, state=state: norm_pair_finish(gp, state, ut_pairs)
                        )
                for ext in extras.values():
                    for t in ext:
                        t()

            # ---------- main emission ----------
            # prologue: exactly what attention qb0-pair0 needs to start; the
            # mask/wv/wf loads queue behind it so they don't delay the first
            # matmuls
            proj_qk_unit("k", 0)
            proj_qk_unit("q", 0)
            proj_qk_unit("q", 1)
            mask_load(0, 0)
            t = consts.tile([128, KC, HD], f16, tag="wv", name="wv")
            nc.sync.dma_start(out=t, in_=wv_d[:])
            w_sb["v"] = t
            for grp in range(1, 4):
                mask_load(0, grp)
            wf_sb = consts.tile([128, 2, D], f16, tag="wf")
            nc.sync.dma_start(out=wf_sb, in_=wf_d[:])

            ut_pairs, deferred = {}, []
            # projection extras run as early as dependencies allow; only the
            # pair-boundary fillers (q-block-1 projections) get their x DMAs
            # issued ahead so their matmuls never wait on HBM
            qbox = [{}, {}]
            vbox = {}
            vbox["x"] = proj_dma("v", 0)
            pipe0 = [
                lambda: proj_v_unit(0, vbox.pop("x")),
                lambda: proj_qk_unit("k", 1),
                lambda: proj_v_unit(1),
                lambda: proj_qk_unit("k", 2),
                lambda: proj_v_unit(2),
                lambda: proj_qk_unit("k", 3),
                lambda: proj_v_unit(3),
                lambda: qbox[0].update(x=proj_dma("q", 2)),
                lambda: qbox[1].update(x=proj_dma("q", 3)),
            ]
            pipe1 = [
                lambda: proj_qk_unit("q", 2, xt2=qbox[0].pop("x")),
                lambda: proj_qk_unit("q", 3, xt2=qbox[1].pop("x")),
            ]
            emit_attention(
                0, {0: pipe0, 1: pipe1}, ut_pairs, deferred,
                prefetch=[lambda grp=grp: mask_load(1, grp) for grp in range(4)],
            )
            uts0 = lambda p: ut_pairs[(0, p)]
            boxes0 = [{} for _ in range(4)]
            fc_halves = [
                lambda g=g, jj=jj: fc_half(0, g, jj, boxes0[g], uts0)
                for g in range(4)
                for jj in range(2)
            ]
            emit_attention(
                1, {0: fc_halves[:5], 1: fc_halves[5:]}, ut_pairs, deferred,
                extra_ok=lambda p, st: p == 1 or st >= 4,
                last=True,
            )
            for t_ in deferred:
                t_()
            uts1 = lambda p: ut_pairs[(1, p)]
            boxes1 = [{} for _ in range(4)]
            for g in range(4):
                for jj in range(2):
                    fc_half(1, g, jj, boxes1[g], uts1)

    nc.compile()
    return nc


def get_nc():
    if "nc" not in _CACHE:
        _CACHE["nc"] = _build()
    return _CACHE["nc"]


def make_in_maps(q, k, v, mask, wq, bq, wk, wv, wf):
    q = np.asarray(q, np.float32)
    k = np.asarray(k, np.float32)
    v = np.asarray(v, np.float32)
    def tile_x(x):
        # [S, D] -> x^T tiled as [128, S/256, KC, 256]:
        # element (c*128+p, j*256+s) -> [p, j, c, s]
        xt = x.T.astype(np.float16).reshape(KC, 128, S // 256, 256)
        return np.ascontiguousarray(xt.transpose(1, 2, 0, 3))

    xqT = [tile_x(q[b]) for b in range(B)]
    xkT = [tile_x(k[b]) for b in range(B)]
    xvT = [tile_x(v[b]) for b in range(B)]
    def tile_mask(m):
        # mask^T [sk, sq] -> [128, NQB, NKT, QB]: (st*128+p, qb*QB+s) -> [p, qb, st, s]
        mt = m.T.astype(np.float16).reshape(NKT, 128, NQB, QB)
        return np.ascontiguousarray(mt.transpose(1, 2, 0, 3))

    mkT = [tile_mask(np.asarray(mask[b])) for b in range(B)]
    wq = np.asarray(wq, np.float16)
    wk = np.asarray(wk, np.float16)
    wv = np.asarray(wv, np.float16)
    wf = np.asarray(wf, np.float16)
    bq = np.asarray(bq, np.float32)
    in_maps = []
    for c in range(NCORES):
        b, hg = c // HPC, c % HPC
        cols = slice(hg * HD, (hg + 1) * HD)
        in_maps.append(
            {
                "xqT": xqT[b],
                "xkT": xkT[b],
                "xvT": xvT[b],
                "wq": np.ascontiguousarray(
                    wq[:, cols].reshape(KC, 128, HD).transpose(1, 0, 2)
                ),
                "wk": np.ascontiguousarray(
                    wk[:, cols].reshape(KC, 128, HD).transpose(1, 0, 2)
                ),
                "wv": np.ascontiguousarray(
                    wv[:, cols].reshape(KC, 128, HD).transpose(1, 0, 2)
                ),
                "bq": np.ascontiguousarray(bq[cols].reshape(2, 128).T),
                # head-pair stacked fc weights: [128 rows of pair p, p, D]
                "wf": np.ascontiguousarray(
                    wf[cols, :].reshape(2, 128, D).transpose(1, 0, 2)
                ),
                "maskT": mkT[b],
            }
        )
    return in_maps


LAST_RESULTS = None


def kernel(q, k, v, mask, wq, bq, wk, wv, bv, wf, bf, **trace_kwargs):
    from concourse.bass_utils import run_bass_kernel_spmd

    global LAST_RESULTS
    nc = get_nc()
    in_maps = make_in_maps(q, k, v, mask, wq, bq, wk, wv, wf)
    res = run_bass_kernel_spmd(
        nc, in_maps, core_ids=list(range(NCORES)), **trace_kwargs
    )
    LAST_RESULTS = res
    out = np.zeros((B, S, D), np.float64)
    for c in range(NCORES):
        out[c // HPC] += res.results[c]["y"].astype(np.float64)
    extra = (
        np.asarray(bv, np.float64) @ np.asarray(wf, np.float64)
        + np.asarray(bf, np.float64)
    )
    out += extra[None, None, :]
    return out.astype(np.float32)


# revision 48
# speedup vs baseline: 1.2016x; 1.0280x over previous
"""Trainium2 Bass kernel: decoder multi-head attention (B=2, S=2048, D=1024, 16 heads).

Sharding: 8 cores = 2 batches x 4 head-groups (4 heads / 256 dims per core).
Per core (batch b, head group hg), all in transposed layouts:
  Q^T = (wq_c)^T @ xq[b]^T + bq_c      [256, 2048]
  K^T = (wk_c)^T @ xk[b]^T             [256, 2048]
  V   = xv[b] @ wv_c                   [2048, 256] token-major (no bias)
  per head h: scores^T[sk,sq] = K_h^T.T @ Q_h^T            (K=64, head pairs
              run concurrently on PE row groups 0-63/64-127)
              P^T = exp(scores^T/8) * mask^T               (fp16)
              [U^T; rowsum] = [V_h | 1].T @ P^T            (ones col -> rowsum)
  per pair p: UT2_p[128,sq] = [U_h0; U_h1] * (1/rowsum)    (recip_approx_fast
              on PSUM row 64 + gpsimd partition_broadcast; no DRAM roundtrip)
  y_partial = sum_p UT2_p.T @ wf2_p                        [2048, 1024]
              (head-pair stacked: contraction 128)
Host: out[b] = sum_hg y_partial + bv @ wf + bf
(v bias folded out: attention rows sum to 1, so attn@(V+bv) = attn@V + bv.)

Scheduling: x tiles DMA'd once per token group (shared by both w row tiles);
all mask DMAs issued up front on the gpsimd queue; PSUM evictions on DVE
(ACT does exp only); q-proj for the second q-block and fc for the previous
q-block ride in the attention pair-boundary bubbles.
"""

import sys

if "/opt/trn_rl_repo" not in sys.path:
    sys.path.insert(0, "/opt/trn_rl_repo")

import numpy as np

B, S, D = 2, 2048, 1024
NH, DK = 16, 64
NCORES = 8
HPC = 4            # heads per core
HD = HPC * DK      # 256 head dims per core
QB = 1024          # q-block (free dim of scores^T tiles)
NQB = S // QB      # 2
NKT = S // 128     # 16 sk tiles
KC = D // 128      # 8 contraction chunks for projections
LAG = 3            # umm trails scores by LAG sk-tiles

_CACHE = {}


def _build():
    import concourse.mybir as mybir
    import concourse.tile as tile
    from concourse import bacc

    f32 = mybir.dt.float32
    f16 = mybir.dt.float16
    EXP = mybir.ActivationFunctionType.Exp

    nc = bacc.Bacc(
        "TRN2",
        target_bir_lowering=False,
        debug=False,
        enable_asserts=False,
        num_devices=NCORES,
    )

    xq_d = nc.dram_tensor("xqT", [128, S // 256, KC, 256], f16, kind="ExternalInput")
    xk_d = nc.dram_tensor("xkT", [128, S // 256, KC, 256], f16, kind="ExternalInput")
    xv_d = nc.dram_tensor("xvT", [128, S // 256, KC, 256], f16, kind="ExternalInput")
    wq_d = nc.dram_tensor("wq", [128, KC, HD], f16, kind="ExternalInput")
    wk_d = nc.dram_tensor("wk", [128, KC, HD], f16, kind="ExternalInput")
    wv_d = nc.dram_tensor("wv", [128, KC, HD], f16, kind="ExternalInput")
    bq_d = nc.dram_tensor("bq", [128, 2], f32, kind="ExternalInput")
    wf_d = nc.dram_tensor("wf", [128, 2, D], f16, kind="ExternalInput")
    mk_d = nc.dram_tensor("maskT", [128, NQB, NKT, QB], f16, kind="ExternalInput")
    y_d = nc.dram_tensor("y", [S, D], f16, kind="ExternalOutput")

    with tile.TileContext(nc) as tc:
        with (
            tc.tile_pool(name="consts", bufs=1) as consts,
            tc.tile_pool(name="qk", bufs=1) as qkp,
            tc.tile_pool(name="maskc", bufs=5) as maskc,
            tc.tile_pool(name="xs", bufs=2) as xsp,
            tc.tile_pool(name="exp", bufs=5) as expp,
            tc.tile_pool(name="pt", bufs=10) as ptp,
            tc.tile_pool(name="usb", bufs=2) as usbp,
            tc.tile_pool(name="rs", bufs=2) as rsp,
            tc.tile_pool(name="bc", bufs=2) as bcp,
            tc.tile_pool(name="ut", bufs=4) as utp,
            tc.tile_pool(name="yo", bufs=2) as yop,
            tc.tile_pool(name="ps_s", bufs=2, space="PSUM") as ps_s,
            tc.tile_pool(name="ps_a", bufs=2, space="PSUM") as ps_a,
        ):
            # ---- constants (k/q first: the prologue needs only those) ----
            w_sb = {}
            for name, dram in (("k", wk_d), ("q", wq_d)):
                t = consts.tile([128, KC, HD], f16, tag=f"w{name}", name=f"w{name}")
                nc.sync.dma_start(out=t, in_=dram[:])
                w_sb[name] = t
            bq_sb = consts.tile([128, 2], f32, tag="bq")
            nc.sync.dma_start(out=bq_sb, in_=bq_d[:])

            # ---- mask tiles: qb0 right after the prologue; qb1 during
            # qb0-pair1 (issued on the gpsimd queue) ----
            mtiles = {}

            def mask_load(qb, grp):
                mg = maskc.tile([128, 4, QB], f16, tag="mask", name=f"m{qb}_{grp}")
                nc.scalar.dma_start(
                    out=mg, in_=mk_d[:][:, qb, 4 * grp : 4 * grp + 4, :]
                )
                mtiles[(qb, grp)] = mg

            # persistent activations
            QT = [qkp.tile([128, S], f16, tag=f"qt{m}", name=f"qt{m}") for m in range(2)]
            KT = [qkp.tile([128, S], f16, tag=f"kt{m}", name=f"kt{m}") for m in range(2)]
            V = [qkp.tile([128, HPC * 65], f16, tag=f"v{st}", name=f"v{st}") for st in range(NKT)]

            # ---------- emission units ----------
            def proj_dma(proj, g):
                """Issue one token group's 1MB x DMA; returns the tile."""
                src_d = {"q": xq_d, "k": xk_d, "v": xv_d}[proj]
                xt2 = xsp.tile([128, 2, KC, 256], f16, tag="xs", name="xt2")
                nc.sync.dma_start(out=xt2, in_=src_d[:][:, 2 * g : 2 * g + 2, :, :])
                return xt2

            def proj_qk_unit(proj, g, ms=(0, 1), xt2=None):
                """One token group (512 tokens) of q/k projection: one 1MB x
                DMA shared by the requested 128-row weight tiles; evictions
                on DVE."""
                if xt2 is None:
                    xt2 = proj_dma(proj, g)
                for m in ms:
                    for jj in range(2):
                        j = 2 * g + jj
                        xt = xt2[:, jj, :, :]
                        ps = ps_s.tile([128, QB], f32, tag="sc", name="ps")
                        for kc in range(KC):
                            nc.tensor.matmul(
                                ps[:, :256],
                                lhsT=w_sb[proj][:, kc, m * 128 : (m + 1) * 128],
                                rhs=xt[:, kc, :],
                                start=(kc == 0),
                                stop=(kc == KC - 1),
                            )
                        dst = (QT if proj == "q" else KT)[m][:, j * 256 : (j + 1) * 256]
                        if proj == "q":
                            nc.vector.tensor_scalar_add(dst, ps[:, :256], bq_sb[:, m : m + 1])
                        else:
                            nc.vector.tensor_copy(dst, ps[:, :256])

            def proj_v_unit(g, xt2=None):
                """One token group of v projection (4 sk tiles, token-major)."""
                if xt2 is None:
                    xt2 = proj_dma("v", g)
                for jj in range(2):
                    for sub in range(2):
                        st = (2 * g + jj) * 2 + sub
                        ps = ps_s.tile([128, QB], f32, tag="sc", name="ps")
                        for kc in range(KC):
                            nc.tensor.matmul(
                                ps[:, :HD],
                                lhsT=xt2[:, jj, kc, sub * 128 : (sub + 1) * 128],
                                rhs=w_sb["v"][:, kc, :],
                                start=(kc == 0),
                                stop=(kc == KC - 1),
                            )
                        vt = V[st]
                        vt3 = vt.rearrange("p (h c) -> p h c", h=HPC)
                        nc.gpsimd.memset(vt3[:, :, 64:65], 1.0)
                        nc.vector.tensor_copy(
                            vt3[:, :, 0:64],
                            ps[:, :HD].rearrange("p (h c) -> p h c", h=HPC),
                        )

            def scores(h, st, qb, pts):
                c, r = h // 2, 64 * (h % 2)
                ps = ps_s.tile([128, QB], f32, tag="sc", name="sc")
                for half in range(2):
                    nc.tensor.matmul(
                        ps[:, half * 512 : (half + 1) * 512],
                        lhsT=KT[c][r : r + 64, st * 128 : (st + 1) * 128],
                        rhs=QT[c][
                            r : r + 64,
                            qb * QB + half * 512 : qb * QB + (half + 1) * 512,
                        ],
                        start=True,
                        stop=True,
                    )
                et = expp.tile([128, QB], f16, tag="exp", name="et")
                nc.scalar.activation(et, ps, EXP, scale=0.125)
                pt = ptp.tile([128, QB], f16, tag="pt", name="pt")
                nc.vector.tensor_mul(pt, et, mtiles[(qb, st // 4)][:, st % 4, :])
                pts[(h, st)] = pt

            def umm(h, st, ups, pts):
                pt = pts.pop((h, st))
                up = ups[h]
                for half in range(2):
                    nc.tensor.matmul(
                        up[0:65, half * 512 : (half + 1) * 512],
                        lhsT=V[st][:, 65 * h : 65 * h + 65],
                        rhs=pt[:, half * 512 : (half + 1) * 512],
                        start=(st == 0),
                        stop=(st == NKT - 1),
                    )

            def norm_pair_evict(pair, ups, state):
                """Pair-end PSUM readout (frees the accumulators fast): U of
                each head into its own base-0 [64, QB] tile, rowsums into
                base-0 [1, QB] tiles. The reciprocal/broadcast/multiply is
                deferred (norm_pair_finish) to the next pair so it never
                blocks the DVE queue."""
                h0, h1 = 2 * pair, 2 * pair + 1
                up0, up1 = ups.pop(h0), ups.pop(h1)
                usb_lo = usbp.tile([64, QB], f32, tag="usb_lo", name="usb_lo")
                nc.vector.tensor_copy(usb_lo, up0[0:64, :])
                usb_hi = usbp.tile([64, QB], f32, tag="usb_hi", name="usb_hi")
                nc.vector.tensor_copy(usb_hi, up1[0:64, :])
                rs_a = rsp.tile([1, QB], f32, tag="rs_a", name="rs_a")
                nc.vector.tensor_copy(rs_a, up0[64:65, :])
                rs_b = rsp.tile([1, QB], f32, tag="rs_b", name="rs_b")
                nc.vector.tensor_copy(rs_b, up1[64:65, :])
                state["t"] = (usb_lo, usb_hi, rs_a, rs_b)

            def norm_pair_finish(gp, state, ut_pairs):
                """All on-chip: in-place approx reciprocal (base-0 custom op),
                gpsimd partition_broadcast into base-0 [64, QB] tiles, two DVE
                multiplies (both inputs base-0; only the output base differs)."""
                usb_lo, usb_hi, rs_a, rs_b = state.pop("t")
                nc.vector.reciprocal_approx_fast(out=rs_a, in_=rs_a)
                nc.vector.reciprocal_approx_fast(out=rs_b, in_=rs_b)
                bc0 = bcp.tile([64, QB], f32, tag="bc0", name="bc0")
                nc.gpsimd.partition_broadcast(bc0, rs_a)
                bc1 = bcp.tile([64, QB], f32, tag="bc1", name="bc1")
                nc.gpsimd.partition_broadcast(bc1, rs_b)
                ut2 = utp.tile([128, QB], f16, tag="ut", name="ut2")
                nc.vector.tensor_mul(ut2[0:64, :], usb_lo, bc0)
                nc.vector.tensor_mul(ut2[64:128, :], usb_hi, bc1)
                ut_pairs[gp] = ut2

            def fc_half(qb, g, jj, yo_box, ut_get):
                # one 128-row fc tile; on jj==1 ships the [256, D] yo tile
                if jj == 0:
                    yo_box["yo"] = yop.tile([128, 2, D], f16, tag="yo", name="yo")
                yo = yo_box["yo"]
                j = 2 * g + jj
                fp = ps_s.tile([128, QB], f32, tag="sc", name="fp")
                for half in range(2):
                    for p in range(2):
                        nc.tensor.matmul(
                            fp[:, half * 512 : (half + 1) * 512],
                            lhsT=ut_get(p)[:, j * 128 : (j + 1) * 128],
                            rhs=wf_sb[:, p, half * 512 : (half + 1) * 512],
                            start=(p == 0),
                            stop=(p == 1),
                        )
                nc.vector.tensor_copy(yo[:, jj, :], fp)
                if jj == 1:
                    nc.sync.dma_start(
                        out=y_d[:][
                            qb * QB + g * 256 : qb * QB + (g + 1) * 256, :
                        ].rearrange("(r p) n -> p r n", p=128),
                        in_=yo_box.pop("yo"),
                    )

            def emit_stream(extras, ut_pairs):
                """All four head-pairs (2 q-blocks x 2 pairs) as one flat
                stream: the scores of pair g+1 interleave with the trailing
                U-matmuls of pair g, so there is no per-pair pipeline drain
                and the PE never idles long enough to re-throttle. Extras
                ((min_k, closure) list, in order) drip in one per step.
                Normalization of pair g is finished LAG+4 steps into pair
                g+1; the last pair finishes at stream end."""
                ups, pts, deferred = {}, {}, []
                for k in range(4 * NKT + LAG):
                    j = k - LAG
                    if j >= 0 and j % NKT == 4 and deferred:
                        deferred.pop(0)()
                    if extras and k >= extras[0][0]:
                        extras.pop(0)[1]()
                    if k < 4 * NKT:
                        pg_s, st_s = divmod(k, NKT)
                        qb_s, pr_s = divmod(pg_s, 2)
                        scores(2 * pr_s, st_s, qb_s, pts)
                        scores(2 * pr_s + 1, st_s, qb_s, pts)
                    if j >= 0:
                        pg_u, st_u = divmod(j, NKT)
                        qb_u, pr_u = divmod(pg_u, 2)
                        h0, h1 = 2 * pr_u, 2 * pr_u + 1
                        if st_u == 0:
                            ups[h0] = ps_a.tile([128, QB], f32, tag="acc", name="upA")
                            ups[h1] = ps_a.tile([128, QB], f32, tag="acc", name="upB")
                        umm(h0, st_u, ups, pts)
                        umm(h1, st_u, ups, pts)
                        if st_u == NKT - 1:
                            state, gp = {}, (qb_u, pr_u)
                            norm_pair_evict(pr_u, ups, state)
                            if pg_u == 3:
                                norm_pair_finish(gp, state, ut_pairs)
                            else:
                                deferred.append(
                                    lambda gp=gp, state=state: norm_pair_finish(
                                        gp, state, ut_pairs
                                    )
                                )
                for _, t in extras:
                    t()

            # ---------- main emission ----------
            # prologue: exactly what attention qb0-pair0 needs to start; the
            # mask/wv/wf loads queue behind it so they don't delay the first
            # matmuls
            proj_qk_unit("k", 0)
            proj_qk_unit("q", 0)
            proj_qk_unit("q", 1)
            mask_load(0, 0)
            t = consts.tile([128, KC, HD], f16, tag="wv", name="wv")
            nc.sync.dma_start(out=t, in_=wv_d[:])
            w_sb["v"] = t
            for grp in range(1, 4):
                mask_load(0, grp)
            wf_sb = consts.tile([128, 2, D], f16, tag="wf")
            nc.sync.dma_start(out=wf_sb, in_=wf_d[:])

            ut_pairs = {}
            # extras: projection units as early as dependencies allow; the
            # q-block-1 projections get their x DMAs issued ahead of their
            # matmuls; qb1 mask loads and qb0's fc ride mid-stream
            qbox = [{}, {}]
            vbox = {"x": proj_dma("v", 0)}
            uts0 = lambda p: ut_pairs[(0, p)]
            boxes0 = [{} for _ in range(4)]
            extras = (
                [
                    (0, lambda: proj_v_unit(0, vbox.pop("x"))),
                    (0, lambda: proj_qk_unit("k", 1)),
                    (0, lambda: proj_v_unit(1)),
                    (0, lambda: proj_qk_unit("k", 2)),
                    (0, lambda: proj_v_unit(2)),
                    (0, lambda: proj_qk_unit("k", 3)),
                    (0, lambda: proj_v_unit(3)),
                    (0, lambda: qbox[0].update(x=proj_dma("q", 2))),
                    (0, lambda: qbox[1].update(x=proj_dma("q", 3))),
                    (12, lambda: proj_qk_unit("q", 2, xt2=qbox[0].pop("x"))),
                    (12, lambda: proj_qk_unit("q", 3, xt2=qbox[1].pop("x"))),
                ]
                + [(14, lambda grp=grp: mask_load(1, grp)) for grp in range(4)]
                + [
                    (41, lambda g=g, jj=jj: fc_half(0, g, jj, boxes0[g], uts0))
                    for g in range(4)
                    for jj in range(2)
                ]
            )
            emit_stream(extras, ut_pairs)
            uts1 = lambda p: ut_pairs[(1, p)]
            boxes1 = [{} for _ in range(4)]
            for g in range(4):
                for jj in range(2):
                    fc_half(1, g, jj, boxes1[g], uts1)

    nc.compile()
    return nc


def get_nc():
    if "nc" not in _CACHE:
        _CACHE["nc"] = _build()
    return _CACHE["nc"]


def make_in_maps(q, k, v, mask, wq, bq, wk, wv, wf):
    q = np.asarray(q, np.float32)
    k = np.asarray(k, np.float32)
    v = np.asarray(v, np.float32)
    def tile_x(x):
        # [S, D] -> x^T tiled as [128, S/256, KC, 256]:
        # element (c*128+p, j*256+s) -> [p, j, c, s]
        xt = x.T.astype(np.float16).reshape(KC, 128, S // 256, 256)
        return np.ascontiguousarray(xt.transpose(1, 2, 0, 3))

    xqT = [tile_x(q[b]) for b in range(B)]
    xkT = [tile_x(k[b]) for b in range(B)]
    xvT = [tile_x(v[b]) for b in range(B)]
    def tile_mask(m):
        # mask^T [sk, sq] -> [128, NQB, NKT, QB]: (st*128+p, qb*QB+s) -> [p, qb, st, s]
        mt = m.T.astype(np.float16).reshape(NKT, 128, NQB, QB)
        return np.ascontiguousarray(mt.transpose(1, 2, 0, 3))

    mkT = [tile_mask(np.asarray(mask[b])) for b in range(B)]
    wq = np.asarray(wq, np.float16)
    wk = np.asarray(wk, np.float16)
    wv = np.asarray(wv, np.float16)
    wf = np.asarray(wf, np.float16)
    bq = np.asarray(bq, np.float32)
    in_maps = []
    for c in range(NCORES):
        b, hg = c // HPC, c % HPC
        cols = slice(hg * HD, (hg + 1) * HD)
        in_maps.append(
            {
                "xqT": xqT[b],
                "xkT": xkT[b],
                "xvT": xvT[b],
                "wq": np.ascontiguousarray(
                    wq[:, cols].reshape(KC, 128, HD).transpose(1, 0, 2)
                ),
                "wk": np.ascontiguousarray(
                    wk[:, cols].reshape(KC, 128, HD).transpose(1, 0, 2)
                ),
                "wv": np.ascontiguousarray(
                    wv[:, cols].reshape(KC, 128, HD).transpose(1, 0, 2)
                ),
                "bq": np.ascontiguousarray(bq[cols].reshape(2, 128).T),
                # head-pair stacked fc weights: [128 rows of pair p, p, D]
                "wf": np.ascontiguousarray(
                    wf[cols, :].reshape(2, 128, D).transpose(1, 0, 2)
                ),
                "maskT": mkT[b],
            }
        )
    return in_maps


LAST_RESULTS = None


def kernel(q, k, v, mask, wq, bq, wk, wv, bv, wf, bf, **trace_kwargs):
    from concourse.bass_utils import run_bass_kernel_spmd

    global LAST_RESULTS
    nc = get_nc()
    in_maps = make_in_maps(q, k, v, mask, wq, bq, wk, wv, wf)
    res = run_bass_kernel_spmd(
        nc, in_maps, core_ids=list(range(NCORES)), **trace_kwargs
    )
    LAST_RESULTS = res
    out = np.zeros((B, S, D), np.float64)
    for c in range(NCORES):
        out[c // HPC] += res.results[c]["y"].astype(np.float64)
    extra = (
        np.asarray(bv, np.float64) @ np.asarray(wf, np.float64)
        + np.asarray(bf, np.float64)
    )
    out += extra[None, None, :]
    return out.astype(np.float32)


# revision 54
# speedup vs baseline: 1.2273x; 1.0214x over previous
"""Trainium2 Bass kernel: decoder multi-head attention (B=2, S=2048, D=1024, 16 heads).

Sharding: 8 cores = 2 batches x 4 head-groups (4 heads / 256 dims per core).
Per core (batch b, head group hg), all in transposed layouts:
  Q^T = (wq_c)^T @ xq[b]^T + bq_c      [256, 2048]
  K^T = (wk_c)^T @ xk[b]^T             [256, 2048]
  V   = xv[b] @ wv_c                   [2048, 256] token-major (no bias)
  per head h: scores^T[sk,sq] = K_h^T.T @ Q_h^T            (K=64, head pairs
              run concurrently on PE row groups 0-63/64-127)
              P^T = exp(scores^T/8) * mask^T               (fp16)
              [U^T; rowsum] = [V_h | 1].T @ P^T            (ones col -> rowsum)
  per pair p: UT2_p[128,sq] = [U_h0; U_h1] * (1/rowsum)    (recip_approx_fast
              on PSUM row 64 + gpsimd partition_broadcast; no DRAM roundtrip)
  y_partial = sum_p UT2_p.T @ wf2_p                        [2048, 1024]
              (head-pair stacked: contraction 128)
Host: out[b] = sum_hg y_partial + bv @ wf + bf
(v bias folded out: attention rows sum to 1, so attn@(V+bv) = attn@V + bv.)

Scheduling: x tiles DMA'd once per token group (shared by both w row tiles);
all mask DMAs issued up front on the gpsimd queue; PSUM evictions on DVE
(ACT does exp only); q-proj for the second q-block and fc for the previous
q-block ride in the attention pair-boundary bubbles.
"""

import sys

if "/opt/trn_rl_repo" not in sys.path:
    sys.path.insert(0, "/opt/trn_rl_repo")

import numpy as np

B, S, D = 2, 2048, 1024
NH, DK = 16, 64
NCORES = 8
HPC = 4            # heads per core
HD = HPC * DK      # 256 head dims per core
QB = 1024          # q-block (free dim of scores^T tiles)
NQB = S // QB      # 2
NKT = S // 128     # 16 sk tiles
KC = D // 128      # 8 contraction chunks for projections
LAG = 4            # umm trails scores by LAG sk-tiles

_CACHE = {}


def _build():
    import concourse.mybir as mybir
    import concourse.tile as tile
    from concourse import bacc

    f32 = mybir.dt.float32
    f16 = mybir.dt.float16
    EXP = mybir.ActivationFunctionType.Exp

    nc = bacc.Bacc(
        "TRN2",
        target_bir_lowering=False,
        debug=False,
        enable_asserts=False,
        num_devices=NCORES,
    )

    xq_d = nc.dram_tensor("xqT", [128, S // 256, KC, 256], f16, kind="ExternalInput")
    xk_d = nc.dram_tensor("xkT", [128, S // 256, KC, 256], f16, kind="ExternalInput")
    xv_d = nc.dram_tensor("xvT", [128, S // 256, KC, 256], f16, kind="ExternalInput")
    wq_d = nc.dram_tensor("wq", [128, KC, HD], f16, kind="ExternalInput")
    wk_d = nc.dram_tensor("wk", [128, KC, HD], f16, kind="ExternalInput")
    wv_d = nc.dram_tensor("wv", [128, KC, HD], f16, kind="ExternalInput")
    bq_d = nc.dram_tensor("bq", [128, 2], f32, kind="ExternalInput")
    wf_d = nc.dram_tensor("wf", [128, 2, D], f16, kind="ExternalInput")
    mk_d = nc.dram_tensor("maskT", [128, NQB, NKT, QB], f16, kind="ExternalInput")
    y_d = nc.dram_tensor("y", [S, D], f16, kind="ExternalOutput")

    with tile.TileContext(nc) as tc:
        with (
            tc.tile_pool(name="consts", bufs=1) as consts,
            tc.tile_pool(name="qk", bufs=1) as qkp,
            tc.tile_pool(name="maskc", bufs=20) as maskc,
            tc.tile_pool(name="xs", bufs=2) as xsp,
            tc.tile_pool(name="exp", bufs=5) as expp,
            tc.tile_pool(name="pt", bufs=11) as ptp,
            tc.tile_pool(name="usb", bufs=2) as usbp,
            tc.tile_pool(name="rs", bufs=2) as rsp,
            tc.tile_pool(name="bc", bufs=2) as bcp,
            tc.tile_pool(name="ut", bufs=4) as utp,
            tc.tile_pool(name="yo", bufs=2) as yop,
            tc.tile_pool(name="ps_s", bufs=2, space="PSUM") as ps_s,
            tc.tile_pool(name="ps_a", bufs=2, space="PSUM") as ps_a,
        ):
            # ---- constants (k/q first: the prologue needs only those) ----
            w_sb = {}
            for name, dram in (("k", wk_d), ("q", wq_d)):
                t = consts.tile([128, KC, HD], f16, tag=f"w{name}", name=f"w{name}")
                nc.sync.dma_start(out=t, in_=dram[:])
                w_sb[name] = t
            bq_sb = consts.tile([128, 2], f32, tag="bq")
            nc.sync.dma_start(out=bq_sb, in_=bq_d[:])

            # ---- mask tiles: qb0 right after the prologue; qb1 during
            # qb0-pair1 (issued on the gpsimd queue) ----
            mtiles = {}

            def mask_load(qb, st):
                # contiguous [128, QB] tiles: a strided mask operand drops the
                # DVE multiply out of its packed perf-mode
                mg = maskc.tile([128, QB], f16, tag="mask", name=f"m{qb}_{st}")
                nc.sync.dma_start(out=mg, in_=mk_d[:][:, qb, st, :])
                mtiles[(qb, st)] = mg

            # persistent activations
            QT = [qkp.tile([128, S], f16, tag=f"qt{m}", name=f"qt{m}") for m in range(2)]
            KT = [qkp.tile([128, S], f16, tag=f"kt{m}", name=f"kt{m}") for m in range(2)]
            V = [qkp.tile([128, HPC * 65], f16, tag=f"v{st}", name=f"v{st}") for st in range(NKT)]

            # ---------- emission units ----------
            def proj_dma(proj, g):
                """Issue one token group's 1MB x DMA; returns the tile."""
                src_d = {"q": xq_d, "k": xk_d, "v": xv_d}[proj]
                xt2 = xsp.tile([128, 2, KC, 256], f16, tag="xs", name="xt2")
                nc.sync.dma_start(out=xt2, in_=src_d[:][:, 2 * g : 2 * g + 2, :, :])
                return xt2

            def proj_qk_unit(proj, g, ms=(0, 1), xt2=None):
                """One token group (512 tokens) of q/k projection: one 1MB x
                DMA shared by the requested 128-row weight tiles; evictions
                on DVE."""
                if xt2 is None:
                    xt2 = proj_dma(proj, g)
                for m in ms:
                    for jj in range(2):
                        j = 2 * g + jj
                        xt = xt2[:, jj, :, :]
                        ps = ps_s.tile([128, QB], f32, tag="sc", name="ps")
                        for kc in range(KC):
                            nc.tensor.matmul(
                                ps[:, :256],
                                lhsT=w_sb[proj][:, kc, m * 128 : (m + 1) * 128],
                                rhs=xt[:, kc, :],
                                start=(kc == 0),
                                stop=(kc == KC - 1),
                            )
                        dst = (QT if proj == "q" else KT)[m][:, j * 256 : (j + 1) * 256]
                        if proj == "q":
                            nc.vector.tensor_scalar_add(dst, ps[:, :256], bq_sb[:, m : m + 1])
                        else:
                            nc.vector.tensor_copy(dst, ps[:, :256])

            def proj_v_unit(g, xt2=None):
                """One token group of v projection (4 sk tiles, token-major)."""
                if xt2 is None:
                    xt2 = proj_dma("v", g)
                for jj in range(2):
                    for sub in range(2):
                        st = (2 * g + jj) * 2 + sub
                        ps = ps_s.tile([128, QB], f32, tag="sc", name="ps")
                        for kc in range(KC):
                            nc.tensor.matmul(
                                ps[:, :HD],
                                lhsT=xt2[:, jj, kc, sub * 128 : (sub + 1) * 128],
                                rhs=w_sb["v"][:, kc, :],
                                start=(kc == 0),
                                stop=(kc == KC - 1),
                            )
                        vt = V[st]
                        vt3 = vt.rearrange("p (h c) -> p h c", h=HPC)
                        nc.gpsimd.memset(vt3[:, :, 64:65], 1.0)
                        nc.vector.tensor_copy(
                            vt3[:, :, 0:64],
                            ps[:, :HD].rearrange("p (h c) -> p h c", h=HPC),
                        )

            def scores(h, st, qb, pts):
                c, r = h // 2, 64 * (h % 2)
                ps = ps_s.tile([128, QB], f32, tag="sc", name="sc")
                for half in range(2):
                    nc.tensor.matmul(
                        ps[:, half * 512 : (half + 1) * 512],
                        lhsT=KT[c][r : r + 64, st * 128 : (st + 1) * 128],
                        rhs=QT[c][
                            r : r + 64,
                            qb * QB + half * 512 : qb * QB + (half + 1) * 512,
                        ],
                        start=True,
                        stop=True,
                    )
                et = expp.tile([128, QB], f16, tag="exp", name="et")
                nc.scalar.activation(et, ps, EXP, scale=0.125)
                pt = ptp.tile([128, QB], f16, tag="pt", name="pt")
                nc.vector.tensor_mul(pt, et, mtiles[(qb, st)])
                pts[(h, st)] = pt

            def umm(h, st, ups, pts):
                pt = pts.pop((h, st))
                up = ups[h]
                for half in range(2):
                    nc.tensor.matmul(
                        up[0:65, half * 512 : (half + 1) * 512],
                        lhsT=V[st][:, 65 * h : 65 * h + 65],
                        rhs=pt[:, half * 512 : (half + 1) * 512],
                        start=(st == 0),
                        stop=(st == NKT - 1),
                    )

            def norm_pair_evict(pair, ups, state):
                """Pair-end PSUM readout (frees the accumulators fast): U of
                each head into its own base-0 [64, QB] tile, rowsums into
                base-0 [1, QB] tiles. The reciprocal/broadcast/multiply is
                deferred (norm_pair_finish) to the next pair so it never
                blocks the DVE queue."""
                h0, h1 = 2 * pair, 2 * pair + 1
                up0, up1 = ups.pop(h0), ups.pop(h1)
                usb_lo = usbp.tile([64, QB], f32, tag="usb_lo", name="usb_lo")
                nc.vector.tensor_copy(usb_lo, up0[0:64, :])
                usb_hi = usbp.tile([64, QB], f32, tag="usb_hi", name="usb_hi")
                nc.vector.tensor_copy(usb_hi, up1[0:64, :])
                rs_a = rsp.tile([1, QB], f32, tag="rs_a", name="rs_a")
                nc.vector.tensor_copy(rs_a, up0[64:65, :])
                rs_b = rsp.tile([1, QB], f32, tag="rs_b", name="rs_b")
                nc.vector.tensor_copy(rs_b, up1[64:65, :])
                state["t"] = (usb_lo, usb_hi, rs_a, rs_b)

            def norm_pair_finish(gp, state, ut_pairs):
                """All on-chip: in-place approx reciprocal (base-0 custom op),
                gpsimd partition_broadcast into base-0 [64, QB] tiles, two DVE
                multiplies (both inputs base-0; only the output base differs)."""
                usb_lo, usb_hi, rs_a, rs_b = state.pop("t")
                nc.vector.reciprocal_approx_fast(out=rs_a, in_=rs_a)
                nc.vector.reciprocal_approx_fast(out=rs_b, in_=rs_b)
                bc0 = bcp.tile([64, QB], f32, tag="bc0", name="bc0")
                nc.gpsimd.partition_broadcast(bc0, rs_a)
                bc1 = bcp.tile([64, QB], f32, tag="bc1", name="bc1")
                nc.gpsimd.partition_broadcast(bc1, rs_b)
                ut2 = utp.tile([128, QB], f16, tag="ut", name="ut2")
                nc.vector.tensor_mul(ut2[0:64, :], usb_lo, bc0)
                nc.vector.tensor_mul(ut2[64:128, :], usb_hi, bc1)
                ut_pairs[gp] = ut2

            def fc_half(qb, g, jj, yo_box, ut_get):
                # one 128-row fc tile; on jj==1 ships the [256, D] yo tile
                if jj == 0:
                    yo_box["yo"] = yop.tile([128, 2, D], f16, tag="yo", name="yo")
                yo = yo_box["yo"]
                j = 2 * g + jj
                fp = ps_s.tile([128, QB], f32, tag="sc", name="fp")
                for half in range(2):
                    for p in range(2):
                        nc.tensor.matmul(
                            fp[:, half * 512 : (half + 1) * 512],
                            lhsT=ut_get(p)[:, j * 128 : (j + 1) * 128],
                            rhs=wf_sb[:, p, half * 512 : (half + 1) * 512],
                            start=(p == 0),
                            stop=(p == 1),
                        )
                nc.vector.tensor_copy(yo[:, jj, :], fp)
                if jj == 1:
                    nc.sync.dma_start(
                        out=y_d[:][
                            qb * QB + g * 256 : qb * QB + (g + 1) * 256, :
                        ].rearrange("(r p) n -> p r n", p=128),
                        in_=yo_box.pop("yo"),
                    )

            def emit_stream(extras, ut_pairs):
                """All four head-pairs (2 q-blocks x 2 pairs) as one flat
                stream: the scores of pair g+1 interleave with the trailing
                U-matmuls of pair g, so there is no per-pair pipeline drain
                and the PE never idles long enough to re-throttle. Extras
                ((min_k, closure) list, in order) drip in one per step.
                Normalization of pair g is finished LAG+4 steps into pair
                g+1; the last pair finishes at stream end."""
                ups, pts, deferred = {}, {}, []
                for k in range(4 * NKT + LAG):
                    j = k - LAG
                    if j >= 0 and j % NKT == 4 and deferred:
                        deferred.pop(0)()
                    if extras and k >= extras[0][0]:
                        extras.pop(0)[1]()
                    if k < 4 * NKT:
                        pg_s, st_s = divmod(k, NKT)
                        qb_s, pr_s = divmod(pg_s, 2)
                        scores(2 * pr_s, st_s, qb_s, pts)
                        scores(2 * pr_s + 1, st_s, qb_s, pts)
                    if j >= 0:
                        pg_u, st_u = divmod(j, NKT)
                        qb_u, pr_u = divmod(pg_u, 2)
                        h0, h1 = 2 * pr_u, 2 * pr_u + 1
                        if st_u == 0:
                            ups[h0] = ps_a.tile([128, QB], f32, tag="acc", name="upA")
                            ups[h1] = ps_a.tile([128, QB], f32, tag="acc", name="upB")
                        umm(h0, st_u, ups, pts)
                        umm(h1, st_u, ups, pts)
                        if st_u == NKT - 1:
                            state, gp = {}, (qb_u, pr_u)
                            norm_pair_evict(pr_u, ups, state)
                            if pg_u == 3:
                                norm_pair_finish(gp, state, ut_pairs)
                            else:
                                deferred.append(
                                    lambda gp=gp, state=state: norm_pair_finish(
                                        gp, state, ut_pairs
                                    )
                                )
                for _, t in extras:
                    t()

            # ---------- main emission ----------
            # prologue: exactly what attention qb0-pair0 needs to start; the
            # mask/wv/wf loads queue behind it so they don't delay the first
            # matmuls
            proj_qk_unit("k", 0)
            proj_qk_unit("q", 0)
            proj_qk_unit("q", 1)
            for st in range(4):
                mask_load(0, st)
            t = consts.tile([128, KC, HD], f16, tag="wv", name="wv")
            nc.sync.dma_start(out=t, in_=wv_d[:])
            w_sb["v"] = t
            for st in range(4, NKT):
                mask_load(0, st)
            wf_sb = consts.tile([128, 2, D], f16, tag="wf")
            nc.sync.dma_start(out=wf_sb, in_=wf_d[:])

            ut_pairs = {}
            # extras: projection units as early as dependencies allow; the
            # q-block-1 projections get their x DMAs issued ahead of their
            # matmuls; qb1 mask loads and qb0's fc ride mid-stream
            qbox = [{}, {}]
            vbox = {"x": proj_dma("v", 0)}
            uts0 = lambda p: ut_pairs[(0, p)]
            boxes0 = [{} for _ in range(4)]
            extras = (
                [
                    (0, lambda: proj_v_unit(0, vbox.pop("x"))),
                    (0, lambda: proj_qk_unit("k", 1)),
                    (0, lambda: proj_v_unit(1)),
                    (0, lambda: proj_qk_unit("k", 2)),
                    (0, lambda: proj_v_unit(2)),
                    (0, lambda: proj_qk_unit("k", 3)),
                    (0, lambda: proj_v_unit(3)),
                    (0, lambda: qbox[0].update(x=proj_dma("q", 2))),
                    (0, lambda: qbox[1].update(x=proj_dma("q", 3))),
                    (12, lambda: proj_qk_unit("q", 2, xt2=qbox[0].pop("x"))),
                    (12, lambda: proj_qk_unit("q", 3, xt2=qbox[1].pop("x"))),
                ]
                + [(14, lambda st=st: mask_load(1, st)) for st in range(NKT)]
                + [
                    (41, lambda g=g, jj=jj: fc_half(0, g, jj, boxes0[g], uts0))
                    for g in range(4)
                    for jj in range(2)
                ]
            )
            emit_stream(extras, ut_pairs)
            uts1 = lambda p: ut_pairs[(1, p)]
            boxes1 = [{} for _ in range(4)]
            for g in range(4):
                for jj in range(2):
                    fc_half(1, g, jj, boxes1[g], uts1)

    nc.compile()
    return nc


def get_nc():
    if "nc" not in _CACHE:
        _CACHE["nc"] = _build()
    return _CACHE["nc"]


def make_in_maps(q, k, v, mask, wq, bq, wk, wv, wf):
    q = np.asarray(q, np.float32)
    k = np.asarray(k, np.float32)
    v = np.asarray(v, np.float32)
    def tile_x(x):
        # [S, D] -> x^T tiled as [128, S/256, KC, 256]:
        # element (c*128+p, j*256+s) -> [p, j, c, s]
        xt = x.T.astype(np.float16).reshape(KC, 128, S // 256, 256)
        return np.ascontiguousarray(xt.transpose(1, 2, 0, 3))

    xqT = [tile_x(q[b]) for b in range(B)]
    xkT = [tile_x(k[b]) for b in range(B)]
    xvT = [tile_x(v[b]) for b in range(B)]
    def tile_mask(m):
        # mask^T [sk, sq] -> [128, NQB, NKT, QB]: (st*128+p, qb*QB+s) -> [p, qb, st, s]
        mt = m.T.astype(np.float16).reshape(NKT, 128, NQB, QB)
        return np.ascontiguousarray(mt.transpose(1, 2, 0, 3))

    mkT = [tile_mask(np.asarray(mask[b])) for b in range(B)]
    wq = np.asarray(wq, np.float16)
    wk = np.asarray(wk, np.float16)
    wv = np.asarray(wv, np.float16)
    wf = np.asarray(wf, np.float16)
    bq = np.asarray(bq, np.float32)
    in_maps = []
    for c in range(NCORES):
        b, hg = c // HPC, c % HPC
        cols = slice(hg * HD, (hg + 1) * HD)
        in_maps.append(
            {
                "xqT": xqT[b],
                "xkT": xkT[b],
                "xvT": xvT[b],
                "wq": np.ascontiguousarray(
                    wq[:, cols].reshape(KC, 128, HD).transpose(1, 0, 2)
                ),
                "wk": np.ascontiguousarray(
                    wk[:, cols].reshape(KC, 128, HD).transpose(1, 0, 2)
                ),
                "wv": np.ascontiguousarray(
                    wv[:, cols].reshape(KC, 128, HD).transpose(1, 0, 2)
                ),
                "bq": np.ascontiguousarray(bq[cols].reshape(2, 128).T),
                # head-pair stacked fc weights: [128 rows of pair p, p, D]
                "wf": np.ascontiguousarray(
                    wf[cols, :].reshape(2, 128, D).transpose(1, 0, 2)
                ),
                "maskT": mkT[b],
            }
        )
    return in_maps


LAST_RESULTS = None


def kernel(q, k, v, mask, wq, bq, wk, wv, bv, wf, bf, **trace_kwargs):
    from concourse.bass_utils import run_bass_kernel_spmd

    global LAST_RESULTS
    nc = get_nc()
    in_maps = make_in_maps(q, k, v, mask, wq, bq, wk, wv, wf)
    res = run_bass_kernel_spmd(
        nc, in_maps, core_ids=list(range(NCORES)), **trace_kwargs
    )
    LAST_RESULTS = res
    out = np.zeros((B, S, D), np.float64)
    for c in range(NCORES):
        out[c // HPC] += res.results[c]["y"].astype(np.float64)
    extra = (
        np.asarray(bv, np.float64) @ np.asarray(wf, np.float64)
        + np.asarray(bf, np.float64)
    )
    out += extra[None, None, :]
    return out.astype(np.float32)
